# revision 8
# baseline (speedup 1.0000x reference)
"""Trainium2 Bass kernel for nn_PromptGenerator (sparse_attention).

Contract: kernel(**inputs) takes FULL inputs (as produced by
reference.setup_inputs) and returns the FULL [32, 196, 1024] f32 output.
Internally shards batch B=32 across 8 NeuronCores (4 per core), weights
replicated.

Math refactoring (exact, value-independent):
  - LayerNorm affine (ln_w, ln_b) and the in-proj bias are folded into the
    QKV weight/bias on the host; the kernel only standardizes (x-mu)*rsqrt.
  - The attention 1/sqrt(128) scale is folded into the q-part of the in-proj
    weight on the host.
  - Stage-2 never materializes k/v1/v2 over the 1225 support tokens:
      score = (q @ Wk) @ s_img^T          [q @ Wk via A = Wq^T Wk]
      o1    = (attn @ s_img) @ Wv1^T      [softmax rows sum to 1]
      o2    = (attn @ s_mask) @ Wv2^T
    and when Wv2 == I (always true for this module), o2 = attn @ s_mask.
  - All bias vectors in this module are zero (asserted after host folding);
    the kernel specializes on that.

Layouts: q,k projections and attention scores are computed feature-major
(via bf16 DMA-transposes of the standardized activations); v is computed
token-major so that attention output lands feature-major without any PE
transposes.  Softmax row sums come for free from the Exp activation's
accum_out; normalization is applied as a per-partition scale where the
token dim sits on partitions.
"""

import sys

sys.path.insert(0, "/opt/trn_rl_repo")

import numpy as np
import ml_dtypes

import concourse.bass as bass
import concourse.mybir as mybir
import concourse.tile as tile
from concourse.bass_utils import run_bass_kernel_spmd
from concourse.vector_clock import ScopedClock

F32 = mybir.dt.float32
BF16 = mybir.dt.bfloat16
AF = mybir.ActivationFunctionType
BF = ml_dtypes.bfloat16

N_CORES = 8
B = 32
BL = B // N_CORES          # batches per core = 4
N = 25                     # support shots
S = 98                     # support seq len
SP = 112                   # S padded to /16 for dma-transpose
E = 1024
H = 8
KT = E // 128              # 8 k-tiles over embed dim
SB = 5                     # seqs per support block
NSEQ = BL * N              # 100 seqs per core
NBLK = NSEQ // SB          # 20 blocks
XW = SB * SP               # 560: feature-major width (112 stride, 32B-aligned)
SQ = 49                    # query img tokens
SQP = 64                   # padded
QW = BL * SQP              # 256
NIJ = N * SQ               # 1225
NIJT = 10                  # nij tiles of 128 (last = 73)
LAST_NIJ = NIJ - 9 * 128   # 73
LAST_NIJP = 80             # padded to /16
EPS = 1e-5


# ------------------------------------------------------------- wait splitting
def _split_multi_waits(nc, max_waits=1):
    """walrus in this env rejects instructions carrying more than one sync
    wait.  Tile's semaphore assignment freely attaches several.  Hoist the
    extra waits onto single-wait NoOps on the same engine, inserted right
    before the instruction (the engine's NX processes its stream in order,
    so this is semantics-preserving)."""
    n_split = 0
    for fn in nc.m.functions:
        for bb in fn.blocks:
            insts = list(bb.instructions)
            need = any(
                i.sync_info is not None and len(i.sync_info.on_wait) > max_waits
                for i in insts)
            if not need:
                continue
            new = []
            for inst in insts:
                si = inst.sync_info
                if si is not None and len(si.on_wait) > max_waits:
                    waits = list(si.on_wait)
                    extra, keep = waits[:-max_waits], waits[-max_waits:]
                    for j, w in enumerate(extra):
                        nop = mybir.InstNoOp(
                            name=f"{inst.name}-w{j}",
                            engine=inst.engine,
                            bass_nofuse=True,
                            sync_info=mybir.SyncInfo(on_wait=[w], on_update=[]),
                        )
                        nc.register_instruction(nop)
                        new.append(nop)
                    inst.sync_info = mybir.SyncInfo(
                        on_wait=keep, on_update=list(si.on_update))
                    n_split += 1
                new.append(inst)
            bb.instructions = new
    return n_split


# ---------------------------------------------------------------- host prep
def _fold_ln_inproj(ln_w, ln_b, in_w, in_b, q_scale):
    """qkv = ln(x) @ in_w.T + in_b  ->  xh @ W1 + c1 with xh standardized."""
    W1 = (ln_w[:, None] * in_w.T).astype(np.float32)          # [E, 3E]
    c1 = (ln_b @ in_w.T + in_b).astype(np.float32)            # [3E]
    W1[:, :E] *= q_scale
    c1[:E] *= q_scale
    return W1, c1


def _ktiles(w):  # [E, X] f32 -> [KT, 128, X] bf16
    return np.ascontiguousarray(w.reshape(KT, 128, -1)).astype(BF)


def _gaussian_bank_np(sigma=1.0):
    x = np.arange(7.0)
    xx, yy = np.meshgrid(x, x, indexing="ij")
    cy = np.arange(7.0)[:, None, None, None]
    cx = np.arange(7.0)[None, :, None, None]
    k = np.exp(-((xx[None, None] - cy) ** 2 + (yy[None, None] - cx) ** 2)
               / (2.0 * sigma ** 2))
    k = k / k.sum(axis=(-2, -1), keepdims=True)
    return k.reshape(49, 49).astype(np.float32)               # [q, ij]


def _prep_weights(inp):
    qs = np.float32(1.0 / np.sqrt(128.0))
    W1s, c1s = _fold_ln_inproj(inp["ln_w"], inp["ln_b"], inp["s_in_w"], inp["s_in_b"], qs)
    W1q, c1q = _fold_ln_inproj(inp["ln_w"], inp["ln_b"], inp["q_in_w"], inp["q_in_b"], qs)
    A = (inp["Wq"].T @ inp["Wk"]).astype(np.float32)          # [E, E]
    g = _gaussian_bank_np(1.0)
    gfull = np.tile(g, (1, N)).astype(np.float32)             # [49, 1225]
    wv2_is_eye = bool(np.array_equal(inp["Wv2"], np.eye(E, dtype=inp["Wv2"].dtype)))

    # all bias-like terms must be zero for this kernel specialization
    for z in (c1s, c1q, inp["s_out_b"], inp["q_out_b"], inp["bq"], inp["bk"],
              inp["bv1"], inp["bv2"]):
        assert not np.any(np.asarray(z)), "nonzero-bias inputs not supported"

    w = {
        "w1s_qk": _ktiles(W1s[:, :2 * E]),
        "w1s_v": _ktiles(W1s[:, 2 * E:]),
        "w2s": _ktiles(inp["s_out_w"].T.astype(np.float32)),
        "w1q_qk": _ktiles(W1q[:, :2 * E]),
        "w1q_v": _ktiles(W1q[:, 2 * E:]),
        "w2q": _ktiles(inp["q_out_w"].T.astype(np.float32)),
        "a_mat": _ktiles(A),
        "wv1": _ktiles(inp["Wv1"].T.astype(np.float32)),
        "gmask": gfull,
    }
    if not wv2_is_eye:
        w["wv2"] = _ktiles(inp["Wv2"].T.astype(np.float32))
    return w, wv2_is_eye


# ---------------------------------------------------------------- builder
def build_program(has_wv2):
    nc = bass.Bass()

    sup_x = nc.declare_dram_parameter("sup_x", [NSEQ, S, E], F32, isOutput=False)
    qf = nc.declare_dram_parameter("qf", [BL, S, E], F32, isOutput=False)
    w1s_qk = nc.declare_dram_parameter("w1s_qk", [KT, 128, 2 * E], BF16, isOutput=False)
    w1s_v = nc.declare_dram_parameter("w1s_v", [KT, 128, E], BF16, isOutput=False)
    w2s = nc.declare_dram_parameter("w2s", [KT, 128, E], BF16, isOutput=False)
    w1q_qk = nc.declare_dram_parameter("w1q_qk", [KT, 128, 2 * E], BF16, isOutput=False)
    w1q_v = nc.declare_dram_parameter("w1q_v", [KT, 128, E], BF16, isOutput=False)
    w2q = nc.declare_dram_parameter("w2q", [KT, 128, E], BF16, isOutput=False)
    a_mat = nc.declare_dram_parameter("a_mat", [KT, 128, E], BF16, isOutput=False)
    wv1 = nc.declare_dram_parameter("wv1", [KT, 128, E], BF16, isOutput=False)
    gmask = nc.declare_dram_parameter("gmask", [SQ, NIJ], F32, isOutput=False)
    wv2 = None
    if has_wv2:
        wv2 = nc.declare_dram_parameter("wv2", [KT, 128, E], BF16, isOutput=False)

    out = nc.declare_dram_parameter("out", [BL, 196, E], F32, isOutput=True)
    # scratch in DRAM (declared as outputs: useful for debugging, cheap)
    sup_img = nc.declare_dram_parameter("sup_img", [NSEQ * SQ, E], BF16, isOutput=True)
    sup_mask = nc.declare_dram_parameter("sup_mask", [NSEQ * SQ, E], BF16, isOutput=True)

    with tile.TileContext(nc) as tc:
        with tc.tile_pool(name="const", bufs=1) as cpool:
            eps_sb = cpool.tile([128, 1], F32, tag="eps")
            nc.gpsimd.memset(eps_sb, EPS)

            _support_phase(nc, tc, sup_x, w1s_qk, w1s_v, w2s, sup_img, sup_mask,
                           eps_sb)
            with tc.tile_pool(name="qpT_pool", bufs=1) as qpT_pool:
                qpT = _query_phase(nc, tc, qf, w1q_qk, w1q_v, w2q, out, eps_sb,
                                   qpT_pool)
                _stage2_phase(nc, tc, qpT, a_mat, wv1, wv2, gmask,
                              sup_img, sup_mask, out)

    _split_multi_waits(nc)
    nc.finalize()
    return nc


def _ln_standardize(nc, pool_small, x_t, xh_t, nrows, eps_sb):
    """xh[:nrows] = (x - mean) * rsqrt(var + eps), bf16 out."""
    stats = pool_small.tile([128, 2, 6], F32, tag="bnst")
    nc.vector.bn_stats(stats[:nrows, 0, :], x_t[:nrows, 0:512])
    nc.vector.bn_stats(stats[:nrows, 1, :], x_t[:nrows, 512:1024])
    mv = pool_small.tile([128, 2], F32, tag="bnmv")
    nc.vector.bn_aggr(mv[:nrows], stats[:nrows])
    std = pool_small.tile([128, 1], F32, tag="std")
    nc.scalar.activation(std[:nrows], mv[:nrows, 1:2], AF.Sqrt, bias=eps_sb[:nrows])
    r = pool_small.tile([128, 1], F32, tag="rstd")
    nc.vector.reciprocal(r[:nrows], std[:nrows])
    nmu = pool_small.tile([128, 1], F32, tag="nmu")
    nc.vector.tensor_scalar_mul(nmu[:nrows], mv[:nrows, 0:1], -1.0)
    nc.vector.tensor_scalar(
        out=xh_t[:nrows], in0=x_t[:nrows], scalar1=nmu[:nrows], scalar2=r[:nrows],
        op0=mybir.AluOpType.add, op1=mybir.AluOpType.mult)


def _support_phase(nc, tc, sup_x, w1s_qk, w1s_v, w2s, sup_img, sup_mask, eps_sb):
    with (
        tc.tile_pool(name="wa", bufs=1) as wpool,
        tc.tile_pool(name="a_x", bufs=7) as x_pool,
        tc.tile_pool(name="a_small", bufs=8) as small,
        tc.tile_pool(name="a_xh", bufs=7) as xh_pool,
        tc.tile_pool(name="a_xhT", bufs=2) as xhT_pool,
        tc.tile_pool(name="a_qkT", bufs=2) as qkT_pool,
        tc.tile_pool(name="a_v", bufs=7) as v_pool,
        tc.tile_pool(name="a_en", bufs=8) as en_pool,
        tc.tile_pool(name="a_eT", bufs=8) as eT_pool,
        tc.tile_pool(name="a_oT", bufs=2) as oT_pool,
        tc.tile_pool(name="a_sup", bufs=4) as sup_pool,
        tc.tile_pool(name="a_ps_qk", bufs=2, space="PSUM") as ps_qk,
        tc.tile_pool(name="a_ps_vs", bufs=2, space="PSUM") as ps_vs,
        tc.tile_pool(name="a_ps_att", bufs=2, space="PSUM") as ps_att,
    ):
        w1qk_sb, w1v_sb, w2_sb = [], [], []
        for k in range(KT):
            t = wpool.tile([128, 2 * E], BF16, tag=f"w1qk{k}")
            nc.sync.dma_start(t, w1s_qk[k])
            w1qk_sb.append(t)
            t = wpool.tile([128, E], BF16, tag=f"w1v{k}")
            nc.sync.dma_start(t, w1s_v[k])
            w1v_sb.append(t)
            t = wpool.tile([128, E], BF16, tag=f"w2{k}")
            nc.sync.dma_start(t, w2s[k])
            w2_sb.append(t)

        for blk in range(NBLK):
            xs, xhs = [], []
            for s in range(SB):
                seq = blk * SB + s
                x_t = x_pool.tile([S, E], F32, tag="x")
                xsrc = sup_x[seq].rearrange("(i j) f -> i j f", j=14)
                nc.sync.dma_start(x_t[0:SQ], xsrc[:, 0:7, :])
                nc.sync.dma_start(x_t[SQ:S], xsrc[:, 7:14, :])
                xs.append(x_t)
                xh_t = xh_pool.tile([SP, E], BF16, tag="xh")
                nc.gpsimd.memset(xh_t[96:SP, :], 0.0)
                _ln_standardize(nc, small, x_t, xh_t, S, eps_sb)
                xhs.append(xh_t)

            # feature-major standardized activations, [KT][128, XW] (112-stride)
            xhT = [xhT_pool.tile([128, XW], BF16, tag=f"xhT{k}", name=f"xhT{k}")
                   for k in range(KT)]
            for s in range(SB):
                for k in range(KT):
                    nc.sync.dma_start_transpose(
                        out=xhT[k][:, s * SP: s * SP + SP],
                        in_=xhs[s][:, k * 128:(k + 1) * 128])

            # q,k projections, feature-major [16][128, XW]
            qkT = []
            for f in range(2 * KT):
                qt = qkT_pool.tile([128, XW], BF16, tag=f"qkT{f}")
                for c0, cw in ((0, 512), (512, XW - 512)):
                    ps = ps_qk.tile([128, 512], F32, tag="psqk")
                    for k in range(KT):
                        nc.tensor.matmul(
                            ps[:, :cw],
                            lhsT=w1qk_sb[k][:, f * 128:(f + 1) * 128],
                            rhs=xhT[k][:, c0:c0 + cw],
                            start=(k == 0), stop=(k == KT - 1))
                    nc.scalar.copy(qt[:, c0:c0 + cw], ps[:, :cw])
                qkT.append(qt)

            # v projection, token-major per seq [SB][S, E]
            vs = []
            for s in range(SB):
                v_t = v_pool.tile([S, E], BF16, tag="v")
                for half in range(2):
                    ps = ps_vs.tile([S, 512], F32, tag="psvs")
                    for k in range(KT):
                        nc.tensor.matmul(
                            ps, lhsT=xhT[k][:, s * SP: s * SP + S],
                            rhs=w1v_sb[k][:, half * 512:(half + 1) * 512],
                            start=(k == 0), stop=(k == KT - 1))
                    nc.scalar.copy(v_t[:, half * 512:(half + 1) * 512], ps)
                vs.append(v_t)

            # attention per (seq, head); oT feature-major [H][128, SB*S]
            oT = [oT_pool.tile([128, SB * S], BF16, tag=f"oT{h}", name=f"oT{h}")
                  for h in range(H)]
            for s in range(SB):
                for h in range(H):
                    ps_l = ps_att.tile([128, S], F32, tag="psatt")
                    nc.tensor.matmul(ps_l[:S, :],
                                     lhsT=qkT[h][:, s * SP: s * SP + S],
                                     rhs=qkT[KT + h][:, s * SP: s * SP + S])
                    en = en_pool.tile([SP, 128], BF16, tag="en")
                    racc = small.tile([128, 1], F32, tag="racc")
                    nc.scalar.activation(en[:S, 0:S], ps_l[:S, :], AF.Exp,
                                         accum_out=racc[:S])
                    rr = small.tile([128, 1], F32, tag="rr")
                    nc.vector.reciprocal(rr[:S], racc[:S])
                    nc.vector.tensor_scalar_mul(en[:S, 0:S], en[:S, 0:S], rr[:S])
                    eT = eT_pool.tile([128, SP], BF16, tag="eT")
                    nc.sync.dma_start_transpose(out=eT, in_=en)
                    ps_o = ps_att.tile([128, S], F32, tag="psatt")
                    nc.tensor.matmul(ps_o, lhsT=vs[s][:, h * 128:(h + 1) * 128],
                                     rhs=eT[:S, 0:S])
                    nc.vector.tensor_copy(oT[h][:, s * S: s * S + S], ps_o)

            # out-proj (token-major) + residual; split img/mask rows to DRAM
            for s in range(SB):
                seq = blk * SB + s
                sup_bf = sup_pool.tile([S, E], BF16, tag="supbf")
                for half in range(2):
                    ps = ps_vs.tile([S, 512], F32, tag="psvs")
                    for e in range(KT):
                        nc.tensor.matmul(
                            ps, lhsT=oT[e][:, s * S: s * S + S],
                            rhs=w2_sb[e][:, half * 512:(half + 1) * 512],
                            start=(e == 0), stop=(e == KT - 1))
                    nc.vector.tensor_add(
                        sup_bf[:, half * 512:(half + 1) * 512], ps,
                        xs[s][:, half * 512:(half + 1) * 512])
                nc.sync.dma_start(
                    out=sup_img[seq * SQ:(seq + 1) * SQ], in_=sup_bf[0:SQ])
                nc.sync.dma_start(
                    out=sup_mask[seq * SQ:(seq + 1) * SQ], in_=sup_bf[SQ:S])


def _query_phase(nc, tc, qf, w1q_qk, w1q_v, w2q, out, eps_sb, qpT_pool):
    """Query-image self-attention.  Returns qpT (feature-major bf16 tiles)."""
    with (
        tc.tile_pool(name="wb", bufs=1) as wpool,
        tc.tile_pool(name="b_x", bufs=4) as x_pool,
        tc.tile_pool(name="b_small", bufs=8) as small,
        tc.tile_pool(name="b_once", bufs=1) as once,
        tc.tile_pool(name="b_xh", bufs=4) as xh_pool,
        tc.tile_pool(name="b_en", bufs=8) as en_pool,
        tc.tile_pool(name="b_eT", bufs=8) as eT_pool,
        tc.tile_pool(name="b_v", bufs=4) as v_pool,
        tc.tile_pool(name="b_qp", bufs=4) as qp_pool,
        tc.tile_pool(name="b_ps1", bufs=2, space="PSUM") as ps1,
        tc.tile_pool(name="b_ps2", bufs=2, space="PSUM") as ps2,
        tc.tile_pool(name="b_ps3", bufs=2, space="PSUM") as ps3,
    ):
        w1qk_sb, w1v_sb, w2_sb = [], [], []
        for k in range(KT):
            t = wpool.tile([128, 2 * E], BF16, tag=f"w1qk{k}")
            nc.sync.dma_start(t, w1q_qk[k])
            w1qk_sb.append(t)
            t = wpool.tile([128, E], BF16, tag=f"w1v{k}")
            nc.sync.dma_start(t, w1q_v[k])
            w1v_sb.append(t)
            t = wpool.tile([128, E], BF16, tag=f"w2{k}")
            nc.sync.dma_start(t, w2q[k])
            w2_sb.append(t)

        xqs, xhqs = [], []
        for b in range(BL):
            x_t = x_pool.tile([SQ, E], F32, tag="xq")
            nc.sync.dma_start(
                x_t, qf[b].rearrange("(i j) f -> i j f", j=14)[:, 0:7, :])
            xqs.append(x_t)
            xh_t = xh_pool.tile([SQP, E], BF16, tag="xhq")
            nc.gpsimd.memset(xh_t[32:SQP, :], 0.0)
            _ln_standardize(nc, small, x_t, xh_t, SQ, eps_sb)
            xhqs.append(xh_t)

        xhT = [once.tile([128, QW], BF16, tag=f"xhqT{k}", name=f"xhqT{k}")
               for k in range(KT)]
        for b in range(BL):
            for k in range(KT):
                nc.sync.dma_start_transpose(
                    out=xhT[k][:, b * SQP:(b + 1) * SQP],
                    in_=xhqs[b][:, k * 128:(k + 1) * 128])

        qkT = []
        for f in range(2 * KT):
            ps = ps1.tile([128, QW], F32, tag="psq1")
            for k in range(KT):
                nc.tensor.matmul(ps, lhsT=w1qk_sb[k][:, f * 128:(f + 1) * 128],
                                 rhs=xhT[k], start=(k == 0), stop=(k == KT - 1))
            qt = once.tile([128, QW], BF16, tag=f"qkTq{f}")
            nc.scalar.copy(qt, ps)
            qkT.append(qt)

        vqs = []
        for b in range(BL):
            v_t = v_pool.tile([SQ, E], BF16, tag="vq")
            for half in range(2):
                ps = ps2.tile([SQ, 512], F32, tag="psq2")
                for k in range(KT):
                    nc.tensor.matmul(
                        ps, lhsT=xhT[k][:, b * SQP: b * SQP + SQ],
                        rhs=w1v_sb[k][:, half * 512:(half + 1) * 512],
                        start=(k == 0), stop=(k == KT - 1))
                nc.scalar.copy(v_t[:, half * 512:(half + 1) * 512], ps)
            vqs.append(v_t)

        oT = [once.tile([128, BL * SQ], BF16, tag=f"oTq{h}", name=f"oTq{h}")
              for h in range(H)]
        for b in range(BL):
            for h in range(H):
                ps_l = ps3.tile([128, SQ], F32, tag="psq3")
                nc.tensor.matmul(ps_l[:SQ, :],
                                 lhsT=qkT[h][:, b * SQP: b * SQP + SQ],
                                 rhs=qkT[KT + h][:, b * SQP: b * SQP + SQ])
                en = en_pool.tile([SQP, 128], BF16, tag="enq")
                racc = small.tile([128, 1], F32, tag="raccq")
                nc.scalar.activation(en[:SQ, 0:SQ], ps_l[:SQ, :], AF.Exp,
                                     accum_out=racc[:SQ])
                rr = small.tile([128, 1], F32, tag="rrq")
                nc.vector.reciprocal(rr[:SQ], racc[:SQ])
                nc.vector.tensor_scalar_mul(en[:SQ, 0:SQ], en[:SQ, 0:SQ], rr[:SQ])
                eT = eT_pool.tile([128, SQP], BF16, tag="eTq")
                nc.sync.dma_start_transpose(out=eT, in_=en)
                ps_o = ps3.tile([128, SQ], F32, tag="psq3")
                nc.tensor.matmul(ps_o, lhsT=vqs[b][:, h * 128:(h + 1) * 128],
                                 rhs=eT[:SQ, 0:SQ])
                nc.vector.tensor_copy(oT[h][:, b * SQ:(b + 1) * SQ], ps_o)

        qps = []
        for b in range(BL):
            qp = qp_pool.tile([SQ, E], F32, tag="qp")
            for half in range(2):
                ps = ps2.tile([SQ, 512], F32, tag="psq2")
                for e in range(KT):
                    nc.tensor.matmul(
                        ps, lhsT=oT[e][:, b * SQ:(b + 1) * SQ],
                        rhs=w2_sb[e][:, half * 512:(half + 1) * 512],
                        start=(e == 0), stop=(e == KT - 1))
                nc.vector.tensor_add(qp[:, half * 512:(half + 1) * 512], ps,
                                     xqs[b][:, half * 512:(half + 1) * 512])
            qps.append(qp)
            # output: query_tokens img half
            nc.sync.dma_start(
                out=out[b, 98:196].rearrange("(i j) f -> i j f", j=14)[:, 0:7, :],
                in_=qp)
            # query_tokens mask half: copy input -> output via SBUF bounce
            qm = x_pool.tile([SQ, E], F32, tag="qm")
            nc.sync.dma_start(
                qm, qf[b].rearrange("(i j) f -> i j f", j=14)[:, 7:14, :])
            nc.sync.dma_start(
                out=out[b, 98:196].rearrange("(i j) f -> i j f", j=14)[:, 7:14, :],
                in_=qm)

        # qp cast + transpose for stage 2
        qpT = [qpT_pool.tile([128, QW], BF16, tag=f"qpT{k}", name=f"qpT{k}")
               for k in range(KT)]
        for b in range(BL):
            qpb = xh_pool.tile([SQP, E], BF16, tag="qpb")
            nc.gpsimd.memset(qpb[32:SQP, :], 0.0)
            nc.vector.tensor_copy(qpb[:SQ], qps[b])
            for k in range(KT):
                nc.sync.dma_start_transpose(
                    out=qpT[k][:, b * SQP:(b + 1) * SQP],
                    in_=qpb[:, k * 128:(k + 1) * 128])
        return qpT


def _stage2_phase(nc, tc, qpT, a_mat, wv1, wv2, gmask, sup_img, sup_mask, out):
    with (
        tc.tile_pool(name="wc", bufs=1) as wpool,
        tc.tile_pool(name="c_simg", bufs=14) as simg_pool,
        tc.tile_pool(name="c_smask", bufs=14) as smask_pool,
        tc.tile_pool(name="c_sT", bufs=2) as sT_pool,
        tc.tile_pool(name="c_small", bufs=6) as small,
        tc.tile_pool(name="c_e2", bufs=2) as e2_pool,
        tc.tile_pool(name="c_e2T", bufs=12) as e2T_pool,
        tc.tile_pool(name="c_o", bufs=9) as o_pool,
        tc.tile_pool(name="c_ps_sc", bufs=2, space="PSUM") as ps_sc,
        tc.tile_pool(name="c_ps_o2", bufs=2, space="PSUM") as ps_o2p,
        tc.tile_pool(name="c_ps_o1t", bufs=2, space="PSUM") as ps_o1tp,
        tc.tile_pool(name="c_ps_o1", bufs=2, space="PSUM") as ps_o1p,
    ):
        a_sb, wv1_sb, wv2_sb = [], [], []
        for k in range(KT):
            t = wpool.tile([128, E], BF16, tag=f"a{k}")
            nc.sync.dma_start(t, a_mat[k])
            a_sb.append(t)
            t = wpool.tile([128, E], BF16, tag=f"wv1{k}")
            nc.sync.dma_start(t, wv1[k])
            wv1_sb.append(t)
            if wv2 is not None:
                t = wpool.tile([128, E], BF16, tag=f"wv2{k}")
                nc.sync.dma_start(t, wv2[k])
                wv2_sb.append(t)
        gm_sb = wpool.tile([SQ, NIJ], F32, tag="gm")
        nc.sync.dma_start(gm_sb, gmask[:])

        # qk2 = qp @ A, feature-major [KT][128, QW]
        qk2T = []
        for eo in range(KT):
            ps = ps_sc.tile([128, 512], F32, tag="pssc")
            for ei in range(KT):
                nc.tensor.matmul(ps[:, :QW],
                                 lhsT=a_sb[ei][:, eo * 128:(eo + 1) * 128],
                                 rhs=qpT[ei], start=(ei == 0), stop=(ei == KT - 1))
            qt = wpool.tile([128, QW], BF16, tag=f"qk2T{eo}")
            nc.scalar.copy(qt, ps[:, :QW])
            qk2T.append(qt)

        nij_rows = [128] * 9 + [LAST_NIJ]
        chunks = [(0, 512), (512, 512), (1024, NIJ - 1024)]
        for b in range(BL):
            simg_t, smask_t = [], []
            for t_i in range(NIJT):
                rows = nij_rows[t_i]
                prows = 128 if t_i < 9 else LAST_NIJP
                st = simg_pool.tile([prows, E], BF16, tag="simg")
                if t_i == 9:
                    nc.gpsimd.memset(st[64:prows, :], 0.0)
                nc.sync.dma_start(
                    st[:rows],
                    sup_img[b * NIJ + t_i * 128: b * NIJ + t_i * 128 + rows])
                simg_t.append(st)
                st = smask_pool.tile([prows, E], BF16, tag="smask")
                nc.sync.dma_start(
                    st[:rows],
                    sup_mask[b * NIJ + t_i * 128: b * NIJ + t_i * 128 + rows])
                smask_t.append(st)

            # s_imgT feature-major [KT][128, 1280]
            sT = [sT_pool.tile([128, NIJT * 128], BF16, tag=f"sT{k}", name=f"sT{k}")
                  for k in range(KT)]
            for t_i in range(NIJT):
                prows = 128 if t_i < 9 else LAST_NIJP
                for k in range(KT):
                    nc.sync.dma_start_transpose(
                        out=sT[k][:, t_i * 128: t_i * 128 + prows],
                        in_=simg_t[t_i][:, k * 128:(k + 1) * 128])

            # score chunks -> gaussian mask -> exp -> e2 (unnormalized attn)
            e2 = e2_pool.tile([SQP, NIJT * 128], BF16, tag="e2")
            racc3 = small.tile([SQ, 3], F32, tag="racc3")
            for ci, (c0, cw) in enumerate(chunks):
                ps = ps_sc.tile([128, 512], F32, tag="pssc")
                for k in range(KT):
                    nc.tensor.matmul(ps[:SQ, :cw],
                                     lhsT=qk2T[k][:, b * SQP: b * SQP + SQ],
                                     rhs=sT[k][:, c0: c0 + cw],
                                     start=(k == 0), stop=(k == KT - 1))
                msk = small.tile([SQ, 512], F32, tag="msk")
                nc.vector.tensor_mul(msk[:, :cw], ps[:SQ, :cw], gm_sb[:, c0:c0 + cw])
                nc.scalar.activation(e2[:SQ, c0:c0 + cw], msk[:, :cw], AF.Exp,
                                     accum_out=racc3[:, ci:ci + 1])
            r2 = small.tile([SQ, 1], F32, tag="r2")
            nc.vector.reduce_sum(r2, racc3, axis=mybir.AxisListType.X)
            rr2 = small.tile([SQ, 1], F32, tag="rr2")
            nc.vector.reciprocal(rr2, r2)

            e2T = []
            for t_i in range(NIJT):
                et = e2T_pool.tile([128, SQP], BF16, tag="e2T")
                nc.sync.dma_start_transpose(
                    out=et, in_=e2[:, t_i * 128:(t_i + 1) * 128])
                e2T.append(et)

            def _emit_o(src_tiles, proj_sb, col0):
                """o = ((attn @ src) [@ W^T]) * rr2 -> out[b, support rows,
                img cols (col0=0) or mask cols (col0=7)]."""
                if proj_sb is None:
                    for half in range(2):
                        ps = ps_o2p.tile([SQ, 512], F32, tag="pso2")
                        for t_i in range(NIJT):
                            rows = nij_rows[t_i]
                            nc.tensor.matmul(
                                ps, lhsT=e2T[t_i][:rows, 0:SQ],
                                rhs=src_tiles[t_i][:rows,
                                                   half * 512:(half + 1) * 512],
                                start=(t_i == 0), stop=(t_i == NIJT - 1))
                        osb = o_pool.tile([SQ, 512], F32, tag="osb")
                        nc.scalar.activation(osb, ps, AF.Copy, scale=rr2)
                        nc.sync.dma_start(
                            out=out[b, 0:98].rearrange(
                                "(i j) f -> i j f", j=14)[
                                :, col0:col0 + 7, half * 512:(half + 1) * 512],
                            in_=osb)
                else:
                    oT_sb = []
                    for e in range(KT):
                        ps = ps_o1tp.tile([128, SQ], F32, tag="pso1t")
                        for t_i in range(NIJT):
                            rows = nij_rows[t_i]
                            nc.tensor.matmul(
                                ps, lhsT=src_tiles[t_i][:rows,
                                                        e * 128:(e + 1) * 128],
                                rhs=e2T[t_i][:rows, 0:SQ],
                                start=(t_i == 0), stop=(t_i == NIJT - 1))
                        ot = o_pool.tile([128, SQ], BF16, tag="opt")
                        nc.scalar.copy(ot, ps)
                        oT_sb.append(ot)
                    for half in range(2):
                        ps = ps_o1p.tile([SQ, 512], F32, tag="pso1")
                        for e in range(KT):
                            nc.tensor.matmul(
                                ps, lhsT=oT_sb[e],
                                rhs=proj_sb[e][:, half * 512:(half + 1) * 512],
                                start=(e == 0), stop=(e == KT - 1))
                        osb = o_pool.tile([SQ, 512], F32, tag="osb")
                        nc.scalar.activation(osb, ps, AF.Copy, scale=rr2)
                        nc.sync.dma_start(
                            out=out[b, 0:98].rearrange(
                                "(i j) f -> i j f", j=14)[
                                :, col0:col0 + 7, half * 512:(half + 1) * 512],
                            in_=osb)

            _emit_o(simg_t, wv1_sb, 0)
            _emit_o(smask_t, wv2_sb if wv2 is not None else None, 7)


# ---------------------------------------------------------------- entry point
_CACHE = {}


def _get_program(has_wv2):
    if has_wv2 not in _CACHE:
        _CACHE[has_wv2] = build_program(has_wv2)
    return _CACHE[has_wv2]


def kernel(**inputs):
    inputs = {k: np.asarray(v) for k, v in inputs.items()}
    w, wv2_is_eye = _prep_weights(inputs)
    nc = _get_program(has_wv2=not wv2_is_eye)

    support = inputs["support_features"].astype(np.float32)
    query = inputs["query_features"].astype(np.float32)
    in_maps = []
    for c in range(N_CORES):
        m = dict(w)
        m["sup_x"] = np.ascontiguousarray(
            support[c * BL:(c + 1) * BL].reshape(NSEQ, S, E))
        m["qf"] = np.ascontiguousarray(query[c * BL:(c + 1) * BL])
        in_maps.append(m)

    res = run_bass_kernel_spmd(nc, in_maps, list(range(N_CORES)))
    out = np.concatenate([res.results[c]["out"] for c in range(N_CORES)], axis=0)
    return out.reshape(B, 196, E).astype(np.float32)


# revision 11
# speedup vs baseline: 1.3990x; 1.3990x over previous
"""Trainium2 Bass kernel for nn_PromptGenerator (sparse_attention).

Contract: kernel(**inputs) takes FULL inputs (as produced by
reference.setup_inputs) and returns the FULL [32, 196, 1024] f32 output.
Internally shards batch B=32 across 8 NeuronCores (4 per core), weights
replicated.

Math refactoring (exact, value-independent):
  - LayerNorm affine (ln_w, ln_b) and the in-proj bias are folded into the
    QKV weight/bias on the host; the kernel only standardizes (x-mu)*rsqrt.
  - The attention 1/sqrt(128) scale is folded into the q-part of the in-proj
    weight on the host.
  - Stage-2 never materializes k/v1/v2 over the 1225 support tokens:
      score = (q @ Wk) @ s_img^T          [q @ Wk via A = Wq^T Wk]
      o1    = (attn @ s_img) @ Wv1^T      [softmax rows sum to 1]
      o2    = (attn @ s_mask) @ Wv2^T
    and when Wv2 == I (always true for this module), o2 = attn @ s_mask.
  - All bias vectors in this module are zero (asserted after host folding);
    the kernel specializes on that.

Layouts: q,k projections and attention scores are computed feature-major
(via bf16 DMA-transposes of the standardized activations); v is computed
token-major so that attention output lands feature-major without any PE
transposes.  Softmax row sums come for free from the Exp activation's
accum_out; normalization is applied as a per-partition scale where the
token dim sits on partitions.
"""

import sys

sys.path.insert(0, "/opt/trn_rl_repo")

import numpy as np
import ml_dtypes

import concourse.bass as bass
import concourse.mybir as mybir
import concourse.tile as tile
from concourse.bass_utils import run_bass_kernel_spmd
from concourse.masks import make_identity

F32 = mybir.dt.float32
BF16 = mybir.dt.bfloat16
AF = mybir.ActivationFunctionType
BF = ml_dtypes.bfloat16

N_CORES = 8
B = 32
BL = B // N_CORES          # batches per core = 4
N = 25                     # support shots
S = 98                     # support seq len
SP = 112                   # S padded to /16 for dma-transpose
E = 1024
H = 8
KT = E // 128              # 8 k-tiles over embed dim
SB = 4                     # seqs per support block
NSEQ = BL * N              # 100 seqs per core
NBLK = NSEQ // SB          # 25 blocks
XW = SB * SP               # 448: feature-major width (112 stride, 32B-aligned)
SQ = 49                    # query img tokens
SQP = 64                   # padded
QW = BL * SQP              # 256
NIJ = N * SQ               # 1225
NIJT = 10                  # nij tiles of 128 (last = 73)
LAST_NIJ = NIJ - 9 * 128   # 73
LAST_NIJP = 80             # padded to /16
EPS = 1e-5


# ------------------------------------------------------------- wait splitting
def _split_multi_waits(nc, max_waits=1):
    """walrus in this env rejects instructions carrying more than one sync
    wait.  Tile's semaphore assignment freely attaches several.  Hoist the
    extra waits onto single-wait NoOps on the same engine, inserted right
    before the instruction (the engine's NX processes its stream in order,
    so this is semantics-preserving)."""
    n_split = 0
    for fn in nc.m.functions:
        for bb in fn.blocks:
            insts = list(bb.instructions)
            need = any(
                i.sync_info is not None and len(i.sync_info.on_wait) > max_waits
                for i in insts)
            if not need:
                continue
            new = []
            for inst in insts:
                si = inst.sync_info
                if si is not None and len(si.on_wait) > max_waits:
                    waits = list(si.on_wait)
                    extra, keep = waits[:-max_waits], waits[-max_waits:]
                    for j, w in enumerate(extra):
                        nop = mybir.InstNoOp(
                            name=f"{inst.name}-w{j}",
                            engine=inst.engine,
                            bass_nofuse=True,
                            sync_info=mybir.SyncInfo(on_wait=[w], on_update=[]),
                        )
                        nc.register_instruction(nop)
                        new.append(nop)
                    inst.sync_info = mybir.SyncInfo(
                        on_wait=keep, on_update=list(si.on_update))
                    n_split += 1
                new.append(inst)
            bb.instructions = new
    return n_split


# ---------------------------------------------------------------- host prep
def _fold_ln_inproj(ln_w, ln_b, in_w, in_b, q_scale):
    """qkv = ln(x) @ in_w.T + in_b  ->  xh @ W1 + c1 with xh standardized."""
    W1 = (ln_w[:, None] * in_w.T).astype(np.float32)          # [E, 3E]
    c1 = (ln_b @ in_w.T + in_b).astype(np.float32)            # [3E]
    W1[:, :E] *= q_scale
    c1[:E] *= q_scale
    return W1, c1


def _ktiles(w):  # [E, X] f32 -> [KT, 128, X] bf16
    return np.ascontiguousarray(w.reshape(KT, 128, -1)).astype(BF)


def _gaussian_bank_np(sigma=1.0):
    x = np.arange(7.0)
    xx, yy = np.meshgrid(x, x, indexing="ij")
    cy = np.arange(7.0)[:, None, None, None]
    cx = np.arange(7.0)[None, :, None, None]
    k = np.exp(-((xx[None, None] - cy) ** 2 + (yy[None, None] - cx) ** 2)
               / (2.0 * sigma ** 2))
    k = k / k.sum(axis=(-2, -1), keepdims=True)
    return k.reshape(49, 49).astype(np.float32)               # [q, ij]


def _prep_weights(inp):
    qs = np.float32(1.0 / np.sqrt(128.0))
    W1s, c1s = _fold_ln_inproj(inp["ln_w"], inp["ln_b"], inp["s_in_w"], inp["s_in_b"], qs)
    W1q, c1q = _fold_ln_inproj(inp["ln_w"], inp["ln_b"], inp["q_in_w"], inp["q_in_b"], qs)
    A = (inp["Wq"].T @ inp["Wk"]).astype(np.float32)          # [E, E]
    g = _gaussian_bank_np(1.0)
    gfull = np.tile(g, (1, N)).astype(np.float32)             # [49, 1225]
    wv2_is_eye = bool(np.array_equal(inp["Wv2"], np.eye(E, dtype=inp["Wv2"].dtype)))

    # all bias-like terms must be zero for this kernel specialization
    for z in (c1s, c1q, inp["s_out_b"], inp["q_out_b"], inp["bq"], inp["bk"],
              inp["bv1"], inp["bv2"]):
        assert not np.any(np.asarray(z)), "nonzero-bias inputs not supported"

    w = {
        "w1s_qk": _ktiles(W1s[:, :2 * E]),
        "w1s_v": _ktiles(W1s[:, 2 * E:]),
        "w2s": _ktiles(inp["s_out_w"].T.astype(np.float32)),
        "w1q_qk": _ktiles(W1q[:, :2 * E]),
        "w1q_v": _ktiles(W1q[:, 2 * E:]),
        "w2q": _ktiles(inp["q_out_w"].T.astype(np.float32)),
        "a_mat": _ktiles(A),
        "wv1": _ktiles(inp["Wv1"].T.astype(np.float32)),
        "gmask": gfull,
    }
    if not wv2_is_eye:
        w["wv2"] = _ktiles(inp["Wv2"].T.astype(np.float32))
    return w, wv2_is_eye


# ---------------------------------------------------------------- builder
def build_program(has_wv2):
    nc = bass.Bass()

    sup_x = nc.declare_dram_parameter("sup_x", [NSEQ, S, E], F32, isOutput=False)
    qf = nc.declare_dram_parameter("qf", [BL, S, E], F32, isOutput=False)
    w1s_qk = nc.declare_dram_parameter("w1s_qk", [KT, 128, 2 * E], BF16, isOutput=False)
    w1s_v = nc.declare_dram_parameter("w1s_v", [KT, 128, E], BF16, isOutput=False)
    w2s = nc.declare_dram_parameter("w2s", [KT, 128, E], BF16, isOutput=False)
    w1q_qk = nc.declare_dram_parameter("w1q_qk", [KT, 128, 2 * E], BF16, isOutput=False)
    w1q_v = nc.declare_dram_parameter("w1q_v", [KT, 128, E], BF16, isOutput=False)
    w2q = nc.declare_dram_parameter("w2q", [KT, 128, E], BF16, isOutput=False)
    a_mat = nc.declare_dram_parameter("a_mat", [KT, 128, E], BF16, isOutput=False)
    wv1 = nc.declare_dram_parameter("wv1", [KT, 128, E], BF16, isOutput=False)
    gmask = nc.declare_dram_parameter("gmask", [SQ, NIJ], F32, isOutput=False)
    wv2 = None
    if has_wv2:
        wv2 = nc.declare_dram_parameter("wv2", [KT, 128, E], BF16, isOutput=False)

    out = nc.declare_dram_parameter("out", [BL, 196, E], F32, isOutput=True)
    # scratch in DRAM (declared as outputs: useful for debugging, cheap)
    sup_img = nc.declare_dram_parameter("sup_img", [NSEQ * SQ, E], BF16, isOutput=True)
    sup_mask = nc.declare_dram_parameter("sup_mask", [NSEQ * SQ, E], BF16, isOutput=True)

    with tile.TileContext(nc) as tc:
        with tc.tile_pool(name="const", bufs=1) as cpool:
            eps_sb = cpool.tile([128, 1], F32, tag="eps")
            nc.gpsimd.memset(eps_sb, EPS)
            ident = cpool.tile([128, 128], BF16, tag="ident")
            make_identity(nc, ident)

            _support_phase(nc, tc, sup_x, w1s_qk, w1s_v, w2s, sup_img, sup_mask,
                           eps_sb, ident)
            with tc.tile_pool(name="qpT_pool", bufs=1) as qpT_pool:
                qpT = _query_phase(nc, tc, qf, w1q_qk, w1q_v, w2q, out, eps_sb,
                                   qpT_pool)
                _stage2_phase(nc, tc, qpT, a_mat, wv1, wv2, gmask,
                              sup_img, sup_mask, out)

    _split_multi_waits(nc)
    nc.finalize()
    return nc


def _ln_standardize(nc, pool_small, x_t, xh_t, nrows, eps_sb):
    """xh[:nrows] = (x - mean) * rsqrt(var + eps), bf16 out."""
    stats = pool_small.tile([128, 2, 6], F32, tag="bnst")
    nc.vector.bn_stats(stats[:nrows, 0, :], x_t[:nrows, 0:512])
    nc.vector.bn_stats(stats[:nrows, 1, :], x_t[:nrows, 512:1024])
    mv = pool_small.tile([128, 2], F32, tag="bnmv")
    nc.vector.bn_aggr(mv[:nrows], stats[:nrows])
    lnv = pool_small.tile([128, 1], F32, tag="lnv")
    nc.scalar.activation(lnv[:nrows], mv[:nrows, 1:2], AF.Ln, bias=eps_sb[:nrows])
    r = pool_small.tile([128, 1], F32, tag="rstd")
    nc.scalar.activation(r[:nrows], lnv[:nrows], AF.Exp, scale=-0.5)
    nmu = pool_small.tile([128, 1], F32, tag="nmu")
    nc.vector.tensor_scalar_mul(nmu[:nrows], mv[:nrows, 0:1], -1.0)
    nc.vector.tensor_scalar(
        out=xh_t[:nrows], in0=x_t[:nrows], scalar1=nmu[:nrows], scalar2=r[:nrows],
        op0=mybir.AluOpType.add, op1=mybir.AluOpType.mult)


def _support_phase(nc, tc, sup_x, w1s_qk, w1s_v, w2s, sup_img, sup_mask, eps_sb,
                   ident):
    with (
        tc.tile_pool(name="wa", bufs=1) as wpool,
        tc.tile_pool(name="a_x", bufs=6) as x_pool,
        tc.tile_pool(name="a_small", bufs=8) as small,
        tc.tile_pool(name="a_xh", bufs=6) as xh_pool,
        tc.tile_pool(name="a_xhT", bufs=2) as xhT_pool,
        tc.tile_pool(name="a_qkT", bufs=2) as qkT_pool,
        tc.tile_pool(name="a_v", bufs=6) as v_pool,
        tc.tile_pool(name="a_en", bufs=10) as en_pool,
        tc.tile_pool(name="a_eT", bufs=10) as eT_pool,
        tc.tile_pool(name="a_oT", bufs=2) as oT_pool,
        tc.tile_pool(name="a_sup", bufs=4) as sup_pool,
        tc.tile_pool(name="a_ps_big", bufs=2, space="PSUM") as ps_big,
        tc.tile_pool(name="a_ps_l", bufs=2, space="PSUM") as ps_l_pool,
        tc.tile_pool(name="a_ps_t", bufs=2, space="PSUM") as ps_t_pool,
        tc.tile_pool(name="a_ps_o", bufs=2, space="PSUM") as ps_o_pool,
    ):
        w1qk_sb, w1v_sb, w2_sb = [], [], []
        for k in range(KT):
            t = wpool.tile([128, 2 * E], BF16, tag=f"w1qk{k}")
            nc.sync.dma_start(t, w1s_qk[k])
            w1qk_sb.append(t)
            t = wpool.tile([128, E], BF16, tag=f"w1v{k}")
            nc.sync.dma_start(t, w1s_v[k])
            w1v_sb.append(t)
            t = wpool.tile([128, E], BF16, tag=f"w2{k}")
            nc.sync.dma_start(t, w2s[k])
            w2_sb.append(t)

        for blk in range(NBLK):
            xs, xhs = [], []
            for s in range(SB):
                seq = blk * SB + s
                x_t = x_pool.tile([S, E], F32, tag="x")
                xsrc = sup_x[seq].rearrange("(i j) f -> i j f", j=14)
                nc.sync.dma_start(x_t[0:SQ], xsrc[:, 0:7, :])
                nc.sync.dma_start(x_t[SQ:S], xsrc[:, 7:14, :])
                xs.append(x_t)
                xh_t = xh_pool.tile([SP, E], BF16, tag="xh")
                nc.gpsimd.memset(xh_t[96:SP, :], 0.0)
                _ln_standardize(nc, small, x_t, xh_t, S, eps_sb)
                xhs.append(xh_t)

            # feature-major standardized activations, [KT][128, XW] (112-stride)
            xhT = [xhT_pool.tile([128, XW], BF16, tag=f"xhT{k}", name=f"xhT{k}")
                   for k in range(KT)]
            for s in range(SB):
                for k in range(KT):
                    nc.sync.dma_start_transpose(
                        out=xhT[k][:, s * SP: s * SP + SP],
                        in_=xhs[s][:, k * 128:(k + 1) * 128])

            # q,k projections, feature-major [16][128, XW]
            qkT = []
            for f in range(2 * KT):
                qt = qkT_pool.tile([128, XW], BF16, tag=f"qkT{f}")
                ps = ps_big.tile([128, XW], F32, tag="big")
                for k in range(KT):
                    nc.tensor.matmul(
                        ps, lhsT=w1qk_sb[k][:, f * 128:(f + 1) * 128],
                        rhs=xhT[k], start=(k == 0), stop=(k == KT - 1))
                nc.scalar.copy(qt, ps)
                qkT.append(qt)

            # v projection, token-major per seq [SB][S, E]
            vs = []
            for s in range(SB):
                v_t = v_pool.tile([S, E], BF16, tag="v")
                for half in range(2):
                    ps = ps_big.tile([S, 512], F32, tag="big")
                    for k in range(KT):
                        nc.tensor.matmul(
                            ps, lhsT=xhT[k][:, s * SP: s * SP + S],
                            rhs=w1v_sb[k][:, half * 512:(half + 1) * 512],
                            start=(k == 0), stop=(k == KT - 1))
                    nc.scalar.copy(v_t[:, half * 512:(half + 1) * 512], ps)
                vs.append(v_t)

            # attention, software-pipelined over seqs so PE never waits on the
            # exp -> normalize -> transpose chain; oT feature-major
            oT = [oT_pool.tile([128, SB * S], BF16, tag=f"oT{h}", name=f"oT{h}")
                  for h in range(H)]

            def emit_logits(s):
                ens = []
                for h in range(H):
                    ps_l = ps_l_pool.tile([128, S], F32, tag="psl")
                    nc.tensor.matmul(ps_l[:S, :],
                                     lhsT=qkT[h][:, s * SP: s * SP + S],
                                     rhs=qkT[KT + h][:, s * SP: s * SP + S])
                    en = en_pool.tile([S, S], BF16, tag="en")
                    racc = small.tile([128, 1], F32, tag="racc")
                    nc.scalar.activation(en, ps_l[:S, :], AF.Exp,
                                         accum_out=racc[:S])
                    rr = small.tile([128, 1], F32, tag="rr")
                    nc.vector.reciprocal(rr[:S], racc[:S])
                    nc.vector.tensor_scalar_mul(en, en, rr[:S])
                    ens.append(en)
                return ens

            def emit_o(s, ens):
                eTs = []
                for h in range(H):
                    ps_t = ps_t_pool.tile([S, S], BF16, tag="pst")
                    nc.tensor.transpose(ps_t, ens[h], ident[:S, :S])
                    eT = eT_pool.tile([S, S], BF16, tag="eT")
                    nc.vector.tensor_copy(eT, ps_t)
                    eTs.append(eT)
                for h in range(H):
                    ps_o = ps_o_pool.tile([128, S], F32, tag="pso")
                    nc.tensor.matmul(ps_o, lhsT=vs[s][:, h * 128:(h + 1) * 128],
                                     rhs=eTs[h])
                    nc.vector.tensor_copy(oT[h][:, s * S: s * S + S], ps_o)

            prev = None
            for s in range(SB):
                cur = emit_logits(s)
                if prev is not None:
                    emit_o(s - 1, prev)
                prev = cur
            emit_o(SB - 1, prev)

            # out-proj (token-major) + residual; img/mask rows to DRAM
            for s in range(SB):
                seq = blk * SB + s
                sup_bf = sup_pool.tile([S, E], BF16, tag="supbf")
                for half in range(2):
                    ps = ps_big.tile([S, 512], F32, tag="big")
                    for e in range(KT):
                        nc.tensor.matmul(
                            ps, lhsT=oT[e][:, s * S: s * S + S],
                            rhs=w2_sb[e][:, half * 512:(half + 1) * 512],
                            start=(e == 0), stop=(e == KT - 1))
                    nc.vector.tensor_add(
                        sup_bf[:, half * 512:(half + 1) * 512], ps,
                        xs[s][:, half * 512:(half + 1) * 512])
                nc.sync.dma_start(
                    out=sup_img[seq * SQ:(seq + 1) * SQ], in_=sup_bf[0:SQ])
                nc.sync.dma_start(
                    out=sup_mask[seq * SQ:(seq + 1) * SQ], in_=sup_bf[SQ:S])


def _query_phase(nc, tc, qf, w1q_qk, w1q_v, w2q, out, eps_sb, qpT_pool):
    """Query-image self-attention.  Returns qpT (feature-major bf16 tiles)."""
    with (
        tc.tile_pool(name="wb", bufs=1) as wpool,
        tc.tile_pool(name="b_x", bufs=4) as x_pool,
        tc.tile_pool(name="b_small", bufs=8) as small,
        tc.tile_pool(name="b_once", bufs=1) as once,
        tc.tile_pool(name="b_xh", bufs=4) as xh_pool,
        tc.tile_pool(name="b_en", bufs=8) as en_pool,
        tc.tile_pool(name="b_eT", bufs=8) as eT_pool,
        tc.tile_pool(name="b_v", bufs=4) as v_pool,
        tc.tile_pool(name="b_qp", bufs=4) as qp_pool,
        tc.tile_pool(name="b_ps1", bufs=2, space="PSUM") as ps1,
        tc.tile_pool(name="b_ps2", bufs=2, space="PSUM") as ps2,
        tc.tile_pool(name="b_ps3", bufs=2, space="PSUM") as ps3,
    ):
        w1qk_sb, w1v_sb, w2_sb = [], [], []
        for k in range(KT):
            t = wpool.tile([128, 2 * E], BF16, tag=f"w1qk{k}")
            nc.sync.dma_start(t, w1q_qk[k])
            w1qk_sb.append(t)
            t = wpool.tile([128, E], BF16, tag=f"w1v{k}")
            nc.sync.dma_start(t, w1q_v[k])
            w1v_sb.append(t)
            t = wpool.tile([128, E], BF16, tag=f"w2{k}")
            nc.sync.dma_start(t, w2q[k])
            w2_sb.append(t)

        xqs, xhqs = [], []
        for b in range(BL):
            x_t = x_pool.tile([SQ, E], F32, tag="xq")
            nc.sync.dma_start(
                x_t, qf[b].rearrange("(i j) f -> i j f", j=14)[:, 0:7, :])
            xqs.append(x_t)
            xh_t = xh_pool.tile([SQP, E], BF16, tag="xhq")
            nc.gpsimd.memset(xh_t[32:SQP, :], 0.0)
            _ln_standardize(nc, small, x_t, xh_t, SQ, eps_sb)
            xhqs.append(xh_t)

        xhT = [once.tile([128, QW], BF16, tag=f"xhqT{k}", name=f"xhqT{k}")
               for k in range(KT)]
        for b in range(BL):
            for k in range(KT):
                nc.sync.dma_start_transpose(
                    out=xhT[k][:, b * SQP:(b + 1) * SQP],
                    in_=xhqs[b][:, k * 128:(k + 1) * 128])

        qkT = []
        for f in range(2 * KT):
            ps = ps1.tile([128, QW], F32, tag="psq1")
            for k in range(KT):
                nc.tensor.matmul(ps, lhsT=w1qk_sb[k][:, f * 128:(f + 1) * 128],
                                 rhs=xhT[k], start=(k == 0), stop=(k == KT - 1))
            qt = once.tile([128, QW], BF16, tag=f"qkTq{f}")
            nc.scalar.copy(qt, ps)
            qkT.append(qt)

        vqs = []
        for b in range(BL):
            v_t = v_pool.tile([SQ, E], BF16, tag="vq")
            for half in range(2):
                ps = ps2.tile([SQ, 512], F32, tag="psq2")
                for k in range(KT):
                    nc.tensor.matmul(
                        ps, lhsT=xhT[k][:, b * SQP: b * SQP + SQ],
                        rhs=w1v_sb[k][:, half * 512:(half + 1) * 512],
                        start=(k == 0), stop=(k == KT - 1))
                nc.scalar.copy(v_t[:, half * 512:(half + 1) * 512], ps)
            vqs.append(v_t)

        oT = [once.tile([128, BL * SQ], BF16, tag=f"oTq{h}", name=f"oTq{h}")
              for h in range(H)]
        for b in range(BL):
            for h in range(H):
                ps_l = ps3.tile([128, SQ], F32, tag="psq3")
                nc.tensor.matmul(ps_l[:SQ, :],
                                 lhsT=qkT[h][:, b * SQP: b * SQP + SQ],
                                 rhs=qkT[KT + h][:, b * SQP: b * SQP + SQ])
                en = en_pool.tile([SQP, 128], BF16, tag="enq")
                racc = small.tile([128, 1], F32, tag="raccq")
                nc.scalar.activation(en[:SQ, 0:SQ], ps_l[:SQ, :], AF.Exp,
                                     accum_out=racc[:SQ])
                rr = small.tile([128, 1], F32, tag="rrq")
                nc.vector.reciprocal(rr[:SQ], racc[:SQ])
                nc.vector.tensor_scalar_mul(en[:SQ, 0:SQ], en[:SQ, 0:SQ], rr[:SQ])
                eT = eT_pool.tile([128, SQP], BF16, tag="eTq")
                nc.sync.dma_start_transpose(out=eT, in_=en)
                ps_o = ps3.tile([128, SQ], F32, tag="psq3")
                nc.tensor.matmul(ps_o, lhsT=vqs[b][:, h * 128:(h + 1) * 128],
                                 rhs=eT[:SQ, 0:SQ])
                nc.vector.tensor_copy(oT[h][:, b * SQ:(b + 1) * SQ], ps_o)

        qps = []
        for b in range(BL):
            qp = qp_pool.tile([SQ, E], F32, tag="qp")
            for half in range(2):
                ps = ps2.tile([SQ, 512], F32, tag="psq2")
                for e in range(KT):
                    nc.tensor.matmul(
                        ps, lhsT=oT[e][:, b * SQ:(b + 1) * SQ],
                        rhs=w2_sb[e][:, half * 512:(half + 1) * 512],
                        start=(e == 0), stop=(e == KT - 1))
                nc.vector.tensor_add(qp[:, half * 512:(half + 1) * 512], ps,
                                     xqs[b][:, half * 512:(half + 1) * 512])
            qps.append(qp)
            # output: query_tokens img half
            nc.sync.dma_start(
                out=out[b, 98:196].rearrange("(i j) f -> i j f", j=14)[:, 0:7, :],
                in_=qp)
            # query_tokens mask half: copy input -> output via SBUF bounce
            qm = x_pool.tile([SQ, E], F32, tag="qm")
            nc.sync.dma_start(
                qm, qf[b].rearrange("(i j) f -> i j f", j=14)[:, 7:14, :])
            nc.sync.dma_start(
                out=out[b, 98:196].rearrange("(i j) f -> i j f", j=14)[:, 7:14, :],
                in_=qm)

        # qp cast + transpose for stage 2
        qpT = [qpT_pool.tile([128, QW], BF16, tag=f"qpT{k}", name=f"qpT{k}")
               for k in range(KT)]
        for b in range(BL):
            qpb = xh_pool.tile([SQP, E], BF16, tag="qpb")
            nc.gpsimd.memset(qpb[32:SQP, :], 0.0)
            nc.vector.tensor_copy(qpb[:SQ], qps[b])
            for k in range(KT):
                nc.sync.dma_start_transpose(
                    out=qpT[k][:, b * SQP:(b + 1) * SQP],
                    in_=qpb[:, k * 128:(k + 1) * 128])
        return qpT


def _stage2_phase(nc, tc, qpT, a_mat, wv1, wv2, gmask, sup_img, sup_mask, out):
    with (
        tc.tile_pool(name="wc", bufs=1) as wpool,
        tc.tile_pool(name="c_simg", bufs=14) as simg_pool,
        tc.tile_pool(name="c_smask", bufs=14) as smask_pool,
        tc.tile_pool(name="c_sT", bufs=2) as sT_pool,
        tc.tile_pool(name="c_small", bufs=6) as small,
        tc.tile_pool(name="c_e2", bufs=2) as e2_pool,
        tc.tile_pool(name="c_e2T", bufs=12) as e2T_pool,
        tc.tile_pool(name="c_o", bufs=9) as o_pool,
        tc.tile_pool(name="c_ps_sc", bufs=2, space="PSUM") as ps_sc,
        tc.tile_pool(name="c_ps_o2", bufs=2, space="PSUM") as ps_o2p,
        tc.tile_pool(name="c_ps_o1t", bufs=2, space="PSUM") as ps_o1tp,
        tc.tile_pool(name="c_ps_o1", bufs=2, space="PSUM") as ps_o1p,
    ):
        a_sb, wv1_sb, wv2_sb = [], [], []
        for k in range(KT):
            t = wpool.tile([128, E], BF16, tag=f"a{k}")
            nc.sync.dma_start(t, a_mat[k])
            a_sb.append(t)
            t = wpool.tile([128, E], BF16, tag=f"wv1{k}")
            nc.sync.dma_start(t, wv1[k])
            wv1_sb.append(t)
            if wv2 is not None:
                t = wpool.tile([128, E], BF16, tag=f"wv2{k}")
                nc.sync.dma_start(t, wv2[k])
                wv2_sb.append(t)
        gm_sb = wpool.tile([SQ, NIJ], F32, tag="gm")
        nc.sync.dma_start(gm_sb, gmask[:])

        # qk2 = qp @ A, feature-major [KT][128, QW]
        qk2T = []
        for eo in range(KT):
            ps = ps_sc.tile([128, 512], F32, tag="pssc")
            for ei in range(KT):
                nc.tensor.matmul(ps[:, :QW],
                                 lhsT=a_sb[ei][:, eo * 128:(eo + 1) * 128],
                                 rhs=qpT[ei], start=(ei == 0), stop=(ei == KT - 1))
            qt = wpool.tile([128, QW], BF16, tag=f"qk2T{eo}")
            nc.scalar.copy(qt, ps[:, :QW])
            qk2T.append(qt)

        nij_rows = [128] * 9 + [LAST_NIJ]
        chunks = [(0, 512), (512, 512), (1024, NIJ - 1024)]
        for b in range(BL):
            simg_t, smask_t = [], []
            for t_i in range(NIJT):
                rows = nij_rows[t_i]
                prows = 128 if t_i < 9 else LAST_NIJP
                st = simg_pool.tile([prows, E], BF16, tag="simg")
                if t_i == 9:
                    nc.gpsimd.memset(st[64:prows, :], 0.0)
                nc.sync.dma_start(
                    st[:rows],
                    sup_img[b * NIJ + t_i * 128: b * NIJ + t_i * 128 + rows])
                simg_t.append(st)
                st = smask_pool.tile([prows, E], BF16, tag="smask")
                nc.sync.dma_start(
                    st[:rows],
                    sup_mask[b * NIJ + t_i * 128: b * NIJ + t_i * 128 + rows])
                smask_t.append(st)

            # s_imgT feature-major [KT][128, 1280]
            sT = [sT_pool.tile([128, NIJT * 128], BF16, tag=f"sT{k}", name=f"sT{k}")
                  for k in range(KT)]
            for t_i in range(NIJT):
                prows = 128 if t_i < 9 else LAST_NIJP
                for k in range(KT):
                    nc.scalar.dma_start_transpose(
                        out=sT[k][:, t_i * 128: t_i * 128 + prows],
                        in_=simg_t[t_i][:, k * 128:(k + 1) * 128])

            # score chunks -> gaussian mask -> exp -> e2 (unnormalized attn)
            e2 = e2_pool.tile([SQP, NIJT * 128], BF16, tag="e2")
            racc3 = small.tile([SQ, 3], F32, tag="racc3")
            for ci, (c0, cw) in enumerate(chunks):
                ps = ps_sc.tile([128, 512], F32, tag="pssc")
                for k in range(KT):
                    nc.tensor.matmul(ps[:SQ, :cw],
                                     lhsT=qk2T[k][:, b * SQP: b * SQP + SQ],
                                     rhs=sT[k][:, c0: c0 + cw],
                                     start=(k == 0), stop=(k == KT - 1))
                msk = small.tile([SQ, 512], F32, tag="msk")
                nc.vector.tensor_mul(msk[:, :cw], ps[:SQ, :cw], gm_sb[:, c0:c0 + cw])
                nc.scalar.activation(e2[:SQ, c0:c0 + cw], msk[:, :cw], AF.Exp,
                                     accum_out=racc3[:, ci:ci + 1])
            r2 = small.tile([SQ, 1], F32, tag="r2")
            nc.vector.reduce_sum(r2, racc3, axis=mybir.AxisListType.X)
            rr2 = small.tile([SQ, 1], F32, tag="rr2")
            nc.vector.reciprocal(rr2, r2)

            e2T = []
            for t_i in range(NIJT):
                et = e2T_pool.tile([128, SQP], BF16, tag="e2T")
                nc.scalar.dma_start_transpose(
                    out=et, in_=e2[:, t_i * 128:(t_i + 1) * 128])
                e2T.append(et)

            def _emit_o(src_tiles, proj_sb, col0):
                """o = ((attn @ src) [@ W^T]) * rr2 -> out[b, support rows,
                img cols (col0=0) or mask cols (col0=7)]."""
                if proj_sb is None:
                    for half in range(2):
                        ps = ps_o2p.tile([SQ, 512], F32, tag="pso2")
                        for t_i in range(NIJT):
                            rows = nij_rows[t_i]
                            nc.tensor.matmul(
                                ps, lhsT=e2T[t_i][:rows, 0:SQ],
                                rhs=src_tiles[t_i][:rows,
                                                   half * 512:(half + 1) * 512],
                                start=(t_i == 0), stop=(t_i == NIJT - 1))
                        osb = o_pool.tile([SQ, 512], F32, tag="osb")
                        nc.scalar.activation(osb, ps, AF.Copy, scale=rr2)
                        nc.sync.dma_start(
                            out=out[b, 0:98].rearrange(
                                "(i j) f -> i j f", j=14)[
                                :, col0:col0 + 7, half * 512:(half + 1) * 512],
                            in_=osb)
                else:
                    oT_sb = []
                    for e in range(KT):
                        ps = ps_o1tp.tile([128, SQ], F32, tag="pso1t")
                        for t_i in range(NIJT):
                            rows = nij_rows[t_i]
                            nc.tensor.matmul(
                                ps, lhsT=src_tiles[t_i][:rows,
                                                        e * 128:(e + 1) * 128],
                                rhs=e2T[t_i][:rows, 0:SQ],
                                start=(t_i == 0), stop=(t_i == NIJT - 1))
                        ot = o_pool.tile([128, SQ], BF16, tag="opt")
                        nc.scalar.copy(ot, ps)
                        oT_sb.append(ot)
                    for half in range(2):
                        ps = ps_o1p.tile([SQ, 512], F32, tag="pso1")
                        for e in range(KT):
                            nc.tensor.matmul(
                                ps, lhsT=oT_sb[e],
                                rhs=proj_sb[e][:, half * 512:(half + 1) * 512],
                                start=(e == 0), stop=(e == KT - 1))
                        osb = o_pool.tile([SQ, 512], F32, tag="osb")
                        nc.scalar.activation(osb, ps, AF.Copy, scale=rr2)
                        nc.sync.dma_start(
                            out=out[b, 0:98].rearrange(
                                "(i j) f -> i j f", j=14)[
                                :, col0:col0 + 7, half * 512:(half + 1) * 512],
                            in_=osb)

            _emit_o(simg_t, wv1_sb, 0)
            _emit_o(smask_t, wv2_sb if wv2 is not None else None, 7)


# ---------------------------------------------------------------- entry point
_CACHE = {}


def _get_program(has_wv2):
    if has_wv2 not in _CACHE:
        _CACHE[has_wv2] = build_program(has_wv2)
    return _CACHE[has_wv2]


def kernel(**inputs):
    inputs = {k: np.asarray(v) for k, v in inputs.items()}
    w, wv2_is_eye = _prep_weights(inputs)
    nc = _get_program(has_wv2=not wv2_is_eye)

    support = inputs["support_features"].astype(np.float32)
    query = inputs["query_features"].astype(np.float32)
    in_maps = []
    for c in range(N_CORES):
        m = dict(w)
        m["sup_x"] = np.ascontiguousarray(
            support[c * BL:(c + 1) * BL].reshape(NSEQ, S, E))
        m["qf"] = np.ascontiguousarray(query[c * BL:(c + 1) * BL])
        in_maps.append(m)

    res = run_bass_kernel_spmd(nc, in_maps, list(range(N_CORES)))
    out = np.concatenate([res.results[c]["out"] for c in range(N_CORES)], axis=0)
    return out.reshape(B, 196, E).astype(np.float32)


# revision 12
# speedup vs baseline: 1.5936x; 1.1391x over previous
"""Trainium2 Bass kernel for nn_PromptGenerator (sparse_attention).

Contract: kernel(**inputs) takes FULL inputs (as produced by
reference.setup_inputs) and returns the FULL [32, 196, 1024] f32 output.
Internally shards batch B=32 across 8 NeuronCores (4 per core), weights
replicated.

Math refactoring (exact, value-independent):
  - LayerNorm affine (ln_w, ln_b) and the in-proj bias are folded into the
    QKV weight/bias on the host; the kernel only standardizes (x-mu)*rsqrt.
  - The attention 1/sqrt(128) scale is folded into the q-part of the in-proj
    weight on the host.
  - Stage-2 never materializes k/v1/v2 over the 1225 support tokens:
      score = (q @ Wk) @ s_img^T          [q @ Wk via A = Wq^T Wk]
      o1    = (attn @ s_img) @ Wv1^T      [softmax rows sum to 1]
      o2    = (attn @ s_mask) @ Wv2^T
    and when Wv2 == I (always true for this module), o2 = attn @ s_mask.
  - All bias vectors in this module are zero (asserted after host folding);
    the kernel specializes on that.

Layouts: q,k projections and attention scores are computed feature-major
(via bf16 DMA-transposes of the standardized activations); v is computed
token-major so that attention output lands feature-major without any PE
transposes.  Softmax row sums come for free from the Exp activation's
accum_out; normalization is applied as a per-partition scale where the
token dim sits on partitions.
"""

import sys

sys.path.insert(0, "/opt/trn_rl_repo")

import numpy as np
import ml_dtypes

import concourse.bass as bass
import concourse.mybir as mybir
import concourse.tile as tile
from concourse.bass_utils import run_bass_kernel_spmd
from concourse.masks import make_identity

F32 = mybir.dt.float32
BF16 = mybir.dt.bfloat16
AF = mybir.ActivationFunctionType
BF = ml_dtypes.bfloat16

N_CORES = 8
B = 32
BL = B // N_CORES          # batches per core = 4
N = 25                     # support shots
S = 98                     # support seq len
SP = 112                   # S padded to /16 for dma-transpose
E = 1024
H = 8
KT = E // 128              # 8 k-tiles over embed dim
SB = 4                     # seqs per support block
NSEQ = BL * N              # 100 seqs per core
NBLK = NSEQ // SB          # 25 blocks
XW = SB * SP               # 448: feature-major width (112 stride, 32B-aligned)
SQ = 49                    # query img tokens
SQP = 64                   # padded
QW = BL * SQP              # 256
NIJ = N * SQ               # 1225
NIJT = 10                  # nij tiles of 128 (last = 73)
LAST_NIJ = NIJ - 9 * 128   # 73
LAST_NIJP = 80             # padded to /16
EPS = 1e-5


# ------------------------------------------------------------- wait splitting
def _split_multi_waits(nc, max_waits=1):
    """walrus in this env rejects instructions carrying more than one sync
    wait.  Tile's semaphore assignment freely attaches several.  Hoist the
    extra waits onto single-wait NoOps on the same engine, inserted right
    before the instruction (the engine's NX processes its stream in order,
    so this is semantics-preserving)."""
    n_split = 0
    for fn in nc.m.functions:
        for bb in fn.blocks:
            insts = list(bb.instructions)
            need = any(
                i.sync_info is not None and len(i.sync_info.on_wait) > max_waits
                for i in insts)
            if not need:
                continue
            new = []
            for inst in insts:
                si = inst.sync_info
                if si is not None and len(si.on_wait) > max_waits:
                    waits = list(si.on_wait)
                    extra, keep = waits[:-max_waits], waits[-max_waits:]
                    for j, w in enumerate(extra):
                        nop = mybir.InstNoOp(
                            name=f"{inst.name}-w{j}",
                            engine=inst.engine,
                            bass_nofuse=True,
                            sync_info=mybir.SyncInfo(on_wait=[w], on_update=[]),
                        )
                        nc.register_instruction(nop)
                        new.append(nop)
                    inst.sync_info = mybir.SyncInfo(
                        on_wait=keep, on_update=list(si.on_update))
                    n_split += 1
                new.append(inst)
            bb.instructions = new
    return n_split


# ---------------------------------------------------------------- host prep
def _fold_ln_inproj(ln_w, ln_b, in_w, in_b, q_scale):
    """qkv = ln(x) @ in_w.T + in_b  ->  xh @ W1 + c1 with xh standardized."""
    W1 = (ln_w[:, None] * in_w.T).astype(np.float32)          # [E, 3E]
    c1 = (ln_b @ in_w.T + in_b).astype(np.float32)            # [3E]
    W1[:, :E] *= q_scale
    c1[:E] *= q_scale
    return W1, c1


def _ktiles(w):  # [E, X] f32 -> [KT, 128, X] bf16
    return np.ascontiguousarray(w.reshape(KT, 128, -1)).astype(BF)


def _gaussian_bank_np(sigma=1.0):
    x = np.arange(7.0)
    xx, yy = np.meshgrid(x, x, indexing="ij")
    cy = np.arange(7.0)[:, None, None, None]
    cx = np.arange(7.0)[None, :, None, None]
    k = np.exp(-((xx[None, None] - cy) ** 2 + (yy[None, None] - cx) ** 2)
               / (2.0 * sigma ** 2))
    k = k / k.sum(axis=(-2, -1), keepdims=True)
    return k.reshape(49, 49).astype(np.float32)               # [q, ij]


def _prep_weights(inp):
    qs = np.float32(1.0 / np.sqrt(128.0))
    W1s, c1s = _fold_ln_inproj(inp["ln_w"], inp["ln_b"], inp["s_in_w"], inp["s_in_b"], qs)
    W1q, c1q = _fold_ln_inproj(inp["ln_w"], inp["ln_b"], inp["q_in_w"], inp["q_in_b"], qs)
    A = (inp["Wq"].T @ inp["Wk"]).astype(np.float32)          # [E, E]
    g = _gaussian_bank_np(1.0)
    gfull = np.tile(g, (1, N)).astype(np.float32)             # [49, 1225]
    wv2_is_eye = bool(np.array_equal(inp["Wv2"], np.eye(E, dtype=inp["Wv2"].dtype)))

    # all bias-like terms must be zero for this kernel specialization
    for z in (c1s, c1q, inp["s_out_b"], inp["q_out_b"], inp["bq"], inp["bk"],
              inp["bv1"], inp["bv2"]):
        assert not np.any(np.asarray(z)), "nonzero-bias inputs not supported"

    w = {
        "w1s_qk": _ktiles(W1s[:, :2 * E]),
        "w1s_v": _ktiles(W1s[:, 2 * E:]),
        "w2s": _ktiles(inp["s_out_w"].T.astype(np.float32)),
        "w1q_qk": _ktiles(W1q[:, :2 * E]),
        "w1q_v": _ktiles(W1q[:, 2 * E:]),
        "w2q": _ktiles(inp["q_out_w"].T.astype(np.float32)),
        "a_mat": _ktiles(A),
        "wv1": _ktiles(inp["Wv1"].T.astype(np.float32)),
        "gmask": gfull,
    }
    if not wv2_is_eye:
        w["wv2"] = _ktiles(inp["Wv2"].T.astype(np.float32))
    return w, wv2_is_eye


# ---------------------------------------------------------------- builder
def build_program(has_wv2):
    nc = bass.Bass()

    sup_x = nc.declare_dram_parameter("sup_x", [NSEQ, S, E], F32, isOutput=False)
    qf = nc.declare_dram_parameter("qf", [BL, S, E], F32, isOutput=False)
    w1s_qk = nc.declare_dram_parameter("w1s_qk", [KT, 128, 2 * E], BF16, isOutput=False)
    w1s_v = nc.declare_dram_parameter("w1s_v", [KT, 128, E], BF16, isOutput=False)
    w2s = nc.declare_dram_parameter("w2s", [KT, 128, E], BF16, isOutput=False)
    w1q_qk = nc.declare_dram_parameter("w1q_qk", [KT, 128, 2 * E], BF16, isOutput=False)
    w1q_v = nc.declare_dram_parameter("w1q_v", [KT, 128, E], BF16, isOutput=False)
    w2q = nc.declare_dram_parameter("w2q", [KT, 128, E], BF16, isOutput=False)
    a_mat = nc.declare_dram_parameter("a_mat", [KT, 128, E], BF16, isOutput=False)
    wv1 = nc.declare_dram_parameter("wv1", [KT, 128, E], BF16, isOutput=False)
    gmask = nc.declare_dram_parameter("gmask", [SQ, NIJ], F32, isOutput=False)
    wv2 = None
    if has_wv2:
        wv2 = nc.declare_dram_parameter("wv2", [KT, 128, E], BF16, isOutput=False)

    out = nc.declare_dram_parameter("out", [BL, 196, E], F32, isOutput=True)
    # scratch in DRAM (declared as outputs: useful for debugging, cheap)
    sup_img = nc.declare_dram_parameter("sup_img", [NSEQ * SQ, E], BF16, isOutput=True)
    sup_mask = nc.declare_dram_parameter("sup_mask", [NSEQ * SQ, E], BF16, isOutput=True)

    with tile.TileContext(nc) as tc:
        with tc.tile_pool(name="const", bufs=1) as cpool:
            eps_sb = cpool.tile([128, 1], F32, tag="eps")
            nc.gpsimd.memset(eps_sb, EPS)
            ident = cpool.tile([128, 128], BF16, tag="ident")
            make_identity(nc, ident)

            _support_phase(nc, tc, sup_x, w1s_qk, w1s_v, w2s, sup_img, sup_mask,
                           eps_sb, ident)
            with tc.tile_pool(name="qpT_pool", bufs=1) as qpT_pool:
                qpT = _query_phase(nc, tc, qf, w1q_qk, w1q_v, w2q, out, eps_sb,
                                   qpT_pool)
                _stage2_phase(nc, tc, qpT, a_mat, wv1, wv2, gmask,
                              sup_img, sup_mask, out)

    _split_multi_waits(nc)
    nc.finalize()
    return nc


def _ln_standardize(nc, pool_small, x_t, xh_t, nrows, eps_sb):
    """xh[:nrows] = (x - mean) * rsqrt(var + eps), bf16 out."""
    stats = pool_small.tile([128, 2, 6], F32, tag="bnst")
    nc.vector.bn_stats(stats[:nrows, 0, :], x_t[:nrows, 0:512])
    nc.vector.bn_stats(stats[:nrows, 1, :], x_t[:nrows, 512:1024])
    mv = pool_small.tile([128, 2], F32, tag="bnmv")
    nc.vector.bn_aggr(mv[:nrows], stats[:nrows])
    lnv = pool_small.tile([128, 1], F32, tag="lnv")
    nc.scalar.activation(lnv[:nrows], mv[:nrows, 1:2], AF.Ln, bias=eps_sb[:nrows])
    r = pool_small.tile([128, 1], F32, tag="rstd")
    nc.scalar.activation(r[:nrows], lnv[:nrows], AF.Exp, scale=-0.5)
    nmu = pool_small.tile([128, 1], F32, tag="nmu")
    nc.vector.tensor_scalar_mul(nmu[:nrows], mv[:nrows, 0:1], -1.0)
    nc.vector.tensor_scalar(
        out=xh_t[:nrows], in0=x_t[:nrows], scalar1=nmu[:nrows], scalar2=r[:nrows],
        op0=mybir.AluOpType.add, op1=mybir.AluOpType.mult)


def _support_phase(nc, tc, sup_x, w1s_qk, w1s_v, w2s, sup_img, sup_mask, eps_sb,
                   ident):
    with (
        tc.tile_pool(name="wa", bufs=1) as wpool,
        tc.tile_pool(name="a_x", bufs=6) as x_pool,
        tc.tile_pool(name="a_small", bufs=8) as small,
        tc.tile_pool(name="a_xh", bufs=6) as xh_pool,
        tc.tile_pool(name="a_xhT", bufs=2) as xhT_pool,
        tc.tile_pool(name="a_qkT", bufs=2) as qkT_pool,
        tc.tile_pool(name="a_v", bufs=6) as v_pool,
        tc.tile_pool(name="a_en", bufs=10) as en_pool,
        tc.tile_pool(name="a_eT", bufs=10) as eT_pool,
        tc.tile_pool(name="a_oT", bufs=2) as oT_pool,
        tc.tile_pool(name="a_sup", bufs=4) as sup_pool,
        tc.tile_pool(name="a_ps_big", bufs=2, space="PSUM") as ps_big,
        tc.tile_pool(name="a_ps_l", bufs=2, space="PSUM") as ps_l_pool,
        tc.tile_pool(name="a_ps_t", bufs=2, space="PSUM") as ps_t_pool,
        tc.tile_pool(name="a_ps_o", bufs=2, space="PSUM") as ps_o_pool,
    ):
        w1qk_sb, w1v_sb, w2_sb = [], [], []
        for k in range(KT):
            t = wpool.tile([128, 2 * E], BF16, tag=f"w1qk{k}")
            nc.sync.dma_start(t, w1s_qk[k])
            w1qk_sb.append(t)
            t = wpool.tile([128, E], BF16, tag=f"w1v{k}")
            nc.sync.dma_start(t, w1s_v[k])
            w1v_sb.append(t)
            t = wpool.tile([128, E], BF16, tag=f"w2{k}")
            nc.sync.dma_start(t, w2s[k])
            w2_sb.append(t)

        for blk in range(NBLK):
            xs, xhs = [], []
            for s in range(SB):
                seq = blk * SB + s
                x_t = x_pool.tile([S, E], F32, tag="x")
                xsrc = sup_x[seq].rearrange("(i j) f -> i j f", j=14)
                nc.sync.dma_start(x_t[0:SQ], xsrc[:, 0:7, :])
                nc.sync.dma_start(x_t[SQ:S], xsrc[:, 7:14, :])
                xs.append(x_t)
                xh_t = xh_pool.tile([SP, E], BF16, tag="xh")
                nc.gpsimd.memset(xh_t[96:SP, :], 0.0)
                _ln_standardize(nc, small, x_t, xh_t, S, eps_sb)
                xhs.append(xh_t)

            # feature-major standardized activations, [KT][128, XW] (112-stride)
            xhT = [xhT_pool.tile([128, XW], BF16, tag=f"xhT{k}", name=f"xhT{k}")
                   for k in range(KT)]
            for s in range(SB):
                for k in range(KT):
                    nc.sync.dma_start_transpose(
                        out=xhT[k][:, s * SP: s * SP + SP],
                        in_=xhs[s][:, k * 128:(k + 1) * 128])

            # q,k projections, feature-major [16][128, XW]
            qkT = []
            for f in range(2 * KT):
                qt = qkT_pool.tile([128, XW], BF16, tag=f"qkT{f}")
                ps = ps_big.tile([128, XW], F32, tag="big")
                for k in range(KT):
                    nc.tensor.matmul(
                        ps, lhsT=w1qk_sb[k][:, f * 128:(f + 1) * 128],
                        rhs=xhT[k], start=(k == 0), stop=(k == KT - 1))
                nc.scalar.copy(qt, ps)
                qkT.append(qt)

            # v projection, token-major per seq [SB][S, E]
            vs = []
            for s in range(SB):
                v_t = v_pool.tile([S, E], BF16, tag="v")
                for half in range(2):
                    ps = ps_big.tile([S, 512], F32, tag="big")
                    for k in range(KT):
                        nc.tensor.matmul(
                            ps, lhsT=xhT[k][:, s * SP: s * SP + S],
                            rhs=w1v_sb[k][:, half * 512:(half + 1) * 512],
                            start=(k == 0), stop=(k == KT - 1))
                    nc.scalar.copy(v_t[:, half * 512:(half + 1) * 512], ps)
                vs.append(v_t)

            # attention, software-pipelined over seqs so PE never waits on
            # the exp -> normalize -> transpose chain.  Logits are batched 4
            # heads per PSUM bank so one Exp covers 4 heads; rowsums via DVE
            # reduce.  The out-proj is interleaved two seqs behind to keep PE
            # duty high through the ACT-bound attention region.
            oT = [oT_pool.tile([128, SB * S], BF16, tag=f"oT{h}", name=f"oT{h}")
                  for h in range(H)]

            def emit_logits(s):
                en4s = []
                for g in range(2):
                    ps_l = ps_l_pool.tile([S, 4 * S], F32, tag="psl")
                    for hh in range(4):
                        h = g * 4 + hh
                        nc.tensor.matmul(
                            ps_l[:, hh * S:(hh + 1) * S],
                            lhsT=qkT[h][:, s * SP: s * SP + S],
                            rhs=qkT[KT + h][:, s * SP: s * SP + S],
                            skip_group_check=True)
                    en4 = en_pool.tile([S, 4 * S], BF16, tag="en")
                    nc.scalar.activation(en4, ps_l, AF.Exp)
                    racc = small.tile([S, 4], F32, tag="racc")
                    nc.vector.reduce_sum(
                        racc, en4.rearrange("p (h j) -> p h j", h=4),
                        axis=mybir.AxisListType.X)
                    rr = small.tile([S, 4], F32, tag="rr")
                    nc.vector.reciprocal(rr, racc)
                    for hh in range(4):
                        nc.vector.tensor_scalar_mul(
                            en4[:, hh * S:(hh + 1) * S],
                            en4[:, hh * S:(hh + 1) * S], rr[:, hh:hh + 1])
                    en4s.append(en4)
                return en4s

            def emit_o(s, en4s):
                eTs = []
                for h in range(H):
                    en = en4s[h // 4][:, (h % 4) * S:(h % 4 + 1) * S]
                    ps_t = ps_t_pool.tile([S, S], BF16, tag="pst")
                    nc.tensor.transpose(ps_t, en, ident[:S, :S])
                    eT = eT_pool.tile([S, S], BF16, tag="eT")
                    nc.vector.tensor_copy(eT, ps_t)
                    eTs.append(eT)
                for h in range(H):
                    ps_o = ps_o_pool.tile([128, S], F32, tag="pso")
                    nc.tensor.matmul(ps_o, lhsT=vs[s][:, h * 128:(h + 1) * 128],
                                     rhs=eTs[h])
                    nc.vector.tensor_copy(oT[h][:, s * S: s * S + S], ps_o)

            def emit_outproj(s):
                seq = blk * SB + s
                sup_bf = sup_pool.tile([S, E], BF16, tag="supbf")
                for half in range(2):
                    ps = ps_big.tile([S, 512], F32, tag="big")
                    for e in range(KT):
                        nc.tensor.matmul(
                            ps, lhsT=oT[e][:, s * S: s * S + S],
                            rhs=w2_sb[e][:, half * 512:(half + 1) * 512],
                            start=(e == 0), stop=(e == KT - 1))
                    nc.vector.tensor_add(
                        sup_bf[:, half * 512:(half + 1) * 512], ps,
                        xs[s][:, half * 512:(half + 1) * 512])
                nc.sync.dma_start(
                    out=sup_img[seq * SQ:(seq + 1) * SQ], in_=sup_bf[0:SQ])
                nc.sync.dma_start(
                    out=sup_mask[seq * SQ:(seq + 1) * SQ], in_=sup_bf[SQ:S])

            prev = None
            for s in range(SB):
                cur = emit_logits(s)
                if prev is not None:
                    emit_o(s - 1, prev)
                if s >= 2:
                    emit_outproj(s - 2)
                prev = cur
            emit_o(SB - 1, prev)
            emit_outproj(SB - 2)
            emit_outproj(SB - 1)


def _query_phase(nc, tc, qf, w1q_qk, w1q_v, w2q, out, eps_sb, qpT_pool):
    """Query-image self-attention.  Returns qpT (feature-major bf16 tiles)."""
    with (
        tc.tile_pool(name="wb", bufs=1) as wpool,
        tc.tile_pool(name="b_x", bufs=4) as x_pool,
        tc.tile_pool(name="b_small", bufs=8) as small,
        tc.tile_pool(name="b_once", bufs=1) as once,
        tc.tile_pool(name="b_xh", bufs=4) as xh_pool,
        tc.tile_pool(name="b_en", bufs=8) as en_pool,
        tc.tile_pool(name="b_eT", bufs=8) as eT_pool,
        tc.tile_pool(name="b_v", bufs=4) as v_pool,
        tc.tile_pool(name="b_qp", bufs=4) as qp_pool,
        tc.tile_pool(name="b_ps1", bufs=2, space="PSUM") as ps1,
        tc.tile_pool(name="b_ps2", bufs=2, space="PSUM") as ps2,
        tc.tile_pool(name="b_ps3", bufs=2, space="PSUM") as ps3,
    ):
        w1qk_sb, w1v_sb, w2_sb = [], [], []
        for k in range(KT):
            t = wpool.tile([128, 2 * E], BF16, tag=f"w1qk{k}")
            nc.sync.dma_start(t, w1q_qk[k])
            w1qk_sb.append(t)
            t = wpool.tile([128, E], BF16, tag=f"w1v{k}")
            nc.sync.dma_start(t, w1q_v[k])
            w1v_sb.append(t)
            t = wpool.tile([128, E], BF16, tag=f"w2{k}")
            nc.sync.dma_start(t, w2q[k])
            w2_sb.append(t)

        xqs, xhqs = [], []
        for b in range(BL):
            x_t = x_pool.tile([SQ, E], F32, tag="xq")
            nc.sync.dma_start(
                x_t, qf[b].rearrange("(i j) f -> i j f", j=14)[:, 0:7, :])
            xqs.append(x_t)
            xh_t = xh_pool.tile([SQP, E], BF16, tag="xhq")
            nc.gpsimd.memset(xh_t[32:SQP, :], 0.0)
            _ln_standardize(nc, small, x_t, xh_t, SQ, eps_sb)
            xhqs.append(xh_t)

        xhT = [once.tile([128, QW], BF16, tag=f"xhqT{k}", name=f"xhqT{k}")
               for k in range(KT)]
        for b in range(BL):
            for k in range(KT):
                nc.sync.dma_start_transpose(
                    out=xhT[k][:, b * SQP:(b + 1) * SQP],
                    in_=xhqs[b][:, k * 128:(k + 1) * 128])

        qkT = []
        for f in range(2 * KT):
            ps = ps1.tile([128, QW], F32, tag="psq1")
            for k in range(KT):
                nc.tensor.matmul(ps, lhsT=w1qk_sb[k][:, f * 128:(f + 1) * 128],
                                 rhs=xhT[k], start=(k == 0), stop=(k == KT - 1))
            qt = once.tile([128, QW], BF16, tag=f"qkTq{f}")
            nc.scalar.copy(qt, ps)
            qkT.append(qt)

        vqs = []
        for b in range(BL):
            v_t = v_pool.tile([SQ, E], BF16, tag="vq")
            for half in range(2):
                ps = ps2.tile([SQ, 512], F32, tag="psq2")
                for k in range(KT):
                    nc.tensor.matmul(
                        ps, lhsT=xhT[k][:, b * SQP: b * SQP + SQ],
                        rhs=w1v_sb[k][:, half * 512:(half + 1) * 512],
                        start=(k == 0), stop=(k == KT - 1))
                nc.scalar.copy(v_t[:, half * 512:(half + 1) * 512], ps)
            vqs.append(v_t)

        oT = [once.tile([128, BL * SQ], BF16, tag=f"oTq{h}", name=f"oTq{h}")
              for h in range(H)]
        for b in range(BL):
            for h in range(H):
                ps_l = ps3.tile([128, SQ], F32, tag="psq3")
                nc.tensor.matmul(ps_l[:SQ, :],
                                 lhsT=qkT[h][:, b * SQP: b * SQP + SQ],
                                 rhs=qkT[KT + h][:, b * SQP: b * SQP + SQ])
                en = en_pool.tile([SQP, 128], BF16, tag="enq")
                racc = small.tile([128, 1], F32, tag="raccq")
                nc.scalar.activation(en[:SQ, 0:SQ], ps_l[:SQ, :], AF.Exp,
                                     accum_out=racc[:SQ])
                rr = small.tile([128, 1], F32, tag="rrq")
                nc.vector.reciprocal(rr[:SQ], racc[:SQ])
                nc.vector.tensor_scalar_mul(en[:SQ, 0:SQ], en[:SQ, 0:SQ], rr[:SQ])
                eT = eT_pool.tile([128, SQP], BF16, tag="eTq")
                nc.sync.dma_start_transpose(out=eT, in_=en)
                ps_o = ps3.tile([128, SQ], F32, tag="psq3")
                nc.tensor.matmul(ps_o, lhsT=vqs[b][:, h * 128:(h + 1) * 128],
                                 rhs=eT[:SQ, 0:SQ])
                nc.vector.tensor_copy(oT[h][:, b * SQ:(b + 1) * SQ], ps_o)

        qps = []
        for b in range(BL):
            qp = qp_pool.tile([SQ, E], F32, tag="qp")
            for half in range(2):
                ps = ps2.tile([SQ, 512], F32, tag="psq2")
                for e in range(KT):
                    nc.tensor.matmul(
                        ps, lhsT=oT[e][:, b * SQ:(b + 1) * SQ],
                        rhs=w2_sb[e][:, half * 512:(half + 1) * 512],
                        start=(e == 0), stop=(e == KT - 1))
                nc.vector.tensor_add(qp[:, half * 512:(half + 1) * 512], ps,
                                     xqs[b][:, half * 512:(half + 1) * 512])
            qps.append(qp)
            # output: query_tokens img half
            nc.sync.dma_start(
                out=out[b, 98:196].rearrange("(i j) f -> i j f", j=14)[:, 0:7, :],
                in_=qp)
            # query_tokens mask half: copy input -> output via SBUF bounce
            qm = x_pool.tile([SQ, E], F32, tag="qm")
            nc.sync.dma_start(
                qm, qf[b].rearrange("(i j) f -> i j f", j=14)[:, 7:14, :])
            nc.sync.dma_start(
                out=out[b, 98:196].rearrange("(i j) f -> i j f", j=14)[:, 7:14, :],
                in_=qm)

        # qp cast + transpose for stage 2
        qpT = [qpT_pool.tile([128, QW], BF16, tag=f"qpT{k}", name=f"qpT{k}")
               for k in range(KT)]
        for b in range(BL):
            qpb = xh_pool.tile([SQP, E], BF16, tag="qpb")
            nc.gpsimd.memset(qpb[32:SQP, :], 0.0)
            nc.vector.tensor_copy(qpb[:SQ], qps[b])
            for k in range(KT):
                nc.sync.dma_start_transpose(
                    out=qpT[k][:, b * SQP:(b + 1) * SQP],
                    in_=qpb[:, k * 128:(k + 1) * 128])
        return qpT


def _stage2_phase(nc, tc, qpT, a_mat, wv1, wv2, gmask, sup_img, sup_mask, out):
    with (
        tc.tile_pool(name="wc", bufs=1) as wpool,
        tc.tile_pool(name="c_simg", bufs=14) as simg_pool,
        tc.tile_pool(name="c_smask", bufs=14) as smask_pool,
        tc.tile_pool(name="c_sT", bufs=2) as sT_pool,
        tc.tile_pool(name="c_small", bufs=6) as small,
        tc.tile_pool(name="c_e2", bufs=2) as e2_pool,
        tc.tile_pool(name="c_e2T", bufs=12) as e2T_pool,
        tc.tile_pool(name="c_o", bufs=9) as o_pool,
        tc.tile_pool(name="c_ps_sc", bufs=2, space="PSUM") as ps_sc,
        tc.tile_pool(name="c_ps_o2", bufs=2, space="PSUM") as ps_o2p,
        tc.tile_pool(name="c_ps_o1t", bufs=2, space="PSUM") as ps_o1tp,
        tc.tile_pool(name="c_ps_o1", bufs=2, space="PSUM") as ps_o1p,
    ):
        a_sb, wv1_sb, wv2_sb = [], [], []
        for k in range(KT):
            t = wpool.tile([128, E], BF16, tag=f"a{k}")
            nc.sync.dma_start(t, a_mat[k])
            a_sb.append(t)
            t = wpool.tile([128, E], BF16, tag=f"wv1{k}")
            nc.sync.dma_start(t, wv1[k])
            wv1_sb.append(t)
            if wv2 is not None:
                t = wpool.tile([128, E], BF16, tag=f"wv2{k}")
                nc.sync.dma_start(t, wv2[k])
                wv2_sb.append(t)
        gm_sb = wpool.tile([SQ, NIJ], F32, tag="gm")
        nc.sync.dma_start(gm_sb, gmask[:])

        # qk2 = qp @ A, feature-major [KT][128, QW]
        qk2T = []
        for eo in range(KT):
            ps = ps_sc.tile([128, 512], F32, tag="pssc")
            for ei in range(KT):
                nc.tensor.matmul(ps[:, :QW],
                                 lhsT=a_sb[ei][:, eo * 128:(eo + 1) * 128],
                                 rhs=qpT[ei], start=(ei == 0), stop=(ei == KT - 1))
            qt = wpool.tile([128, QW], BF16, tag=f"qk2T{eo}")
            nc.scalar.copy(qt, ps[:, :QW])
            qk2T.append(qt)

        nij_rows = [128] * 9 + [LAST_NIJ]
        chunks = [(0, 512), (512, 512), (1024, NIJ - 1024)]
        for b in range(BL):
            simg_t, smask_t = [], []
            for t_i in range(NIJT):
                rows = nij_rows[t_i]
                prows = 128 if t_i < 9 else LAST_NIJP
                st = simg_pool.tile([prows, E], BF16, tag="simg")
                if t_i == 9:
                    nc.gpsimd.memset(st[64:prows, :], 0.0)
                nc.sync.dma_start(
                    st[:rows],
                    sup_img[b * NIJ + t_i * 128: b * NIJ + t_i * 128 + rows])
                simg_t.append(st)
                st = smask_pool.tile([prows, E], BF16, tag="smask")
                nc.sync.dma_start(
                    st[:rows],
                    sup_mask[b * NIJ + t_i * 128: b * NIJ + t_i * 128 + rows])
                smask_t.append(st)

            # s_imgT feature-major [KT][128, 1280]
            sT = [sT_pool.tile([128, NIJT * 128], BF16, tag=f"sT{k}", name=f"sT{k}")
                  for k in range(KT)]
            for t_i in range(NIJT):
                prows = 128 if t_i < 9 else LAST_NIJP
                for k in range(KT):
                    nc.scalar.dma_start_transpose(
                        out=sT[k][:, t_i * 128: t_i * 128 + prows],
                        in_=simg_t[t_i][:, k * 128:(k + 1) * 128])

            # score chunks -> gaussian mask -> exp -> e2 (unnormalized attn)
            e2 = e2_pool.tile([SQP, NIJT * 128], BF16, tag="e2")
            racc3 = small.tile([SQ, 3], F32, tag="racc3")
            for ci, (c0, cw) in enumerate(chunks):
                ps = ps_sc.tile([128, 512], F32, tag="pssc")
                for k in range(KT):
                    nc.tensor.matmul(ps[:SQ, :cw],
                                     lhsT=qk2T[k][:, b * SQP: b * SQP + SQ],
                                     rhs=sT[k][:, c0: c0 + cw],
                                     start=(k == 0), stop=(k == KT - 1))
                msk = small.tile([SQ, 512], F32, tag="msk")
                nc.vector.tensor_mul(msk[:, :cw], ps[:SQ, :cw], gm_sb[:, c0:c0 + cw])
                nc.scalar.activation(e2[:SQ, c0:c0 + cw], msk[:, :cw], AF.Exp,
                                     accum_out=racc3[:, ci:ci + 1])
            r2 = small.tile([SQ, 1], F32, tag="r2")
            nc.vector.reduce_sum(r2, racc3, axis=mybir.AxisListType.X)
            rr2 = small.tile([SQ, 1], F32, tag="rr2")
            nc.vector.reciprocal(rr2, r2)

            e2T = []
            for t_i in range(NIJT):
                et = e2T_pool.tile([128, SQP], BF16, tag="e2T")
                nc.scalar.dma_start_transpose(
                    out=et, in_=e2[:, t_i * 128:(t_i + 1) * 128])
                e2T.append(et)

            def _emit_o(src_tiles, proj_sb, col0):
                """o = ((attn @ src) [@ W^T]) * rr2 -> out[b, support rows,
                img cols (col0=0) or mask cols (col0=7)]."""
                if proj_sb is None:
                    for half in range(2):
                        ps = ps_o2p.tile([SQ, 512], F32, tag="pso2")
                        for t_i in range(NIJT):
                            rows = nij_rows[t_i]
                            nc.tensor.matmul(
                                ps, lhsT=e2T[t_i][:rows, 0:SQ],
                                rhs=src_tiles[t_i][:rows,
                                                   half * 512:(half + 1) * 512],
                                start=(t_i == 0), stop=(t_i == NIJT - 1))
                        osb = o_pool.tile([SQ, 512], F32, tag="osb")
                        nc.scalar.activation(osb, ps, AF.Copy, scale=rr2)
                        nc.sync.dma_start(
                            out=out[b, 0:98].rearrange(
                                "(i j) f -> i j f", j=14)[
                                :, col0:col0 + 7, half * 512:(half + 1) * 512],
                            in_=osb)
                else:
                    oT_sb = []
                    for e in range(KT):
                        ps = ps_o1tp.tile([128, SQ], F32, tag="pso1t")
                        for t_i in range(NIJT):
                            rows = nij_rows[t_i]
                            nc.tensor.matmul(
                                ps, lhsT=src_tiles[t_i][:rows,
                                                        e * 128:(e + 1) * 128],
                                rhs=e2T[t_i][:rows, 0:SQ],
                                start=(t_i == 0), stop=(t_i == NIJT - 1))
                        ot = o_pool.tile([128, SQ], BF16, tag="opt")
                        nc.scalar.copy(ot, ps)
                        oT_sb.append(ot)
                    for half in range(2):
                        ps = ps_o1p.tile([SQ, 512], F32, tag="pso1")
                        for e in range(KT):
                            nc.tensor.matmul(
                                ps, lhsT=oT_sb[e],
                                rhs=proj_sb[e][:, half * 512:(half + 1) * 512],
                                start=(e == 0), stop=(e == KT - 1))
                        osb = o_pool.tile([SQ, 512], F32, tag="osb")
                        nc.scalar.activation(osb, ps, AF.Copy, scale=rr2)
                        nc.sync.dma_start(
                            out=out[b, 0:98].rearrange(
                                "(i j) f -> i j f", j=14)[
                                :, col0:col0 + 7, half * 512:(half + 1) * 512],
                            in_=osb)

            _emit_o(simg_t, wv1_sb, 0)
            _emit_o(smask_t, wv2_sb if wv2 is not None else None, 7)


# ---------------------------------------------------------------- entry point
_CACHE = {}


def _get_program(has_wv2):
    if has_wv2 not in _CACHE:
        _CACHE[has_wv2] = build_program(has_wv2)
    return _CACHE[has_wv2]


def kernel(**inputs):
    inputs = {k: np.asarray(v) for k, v in inputs.items()}
    w, wv2_is_eye = _prep_weights(inputs)
    nc = _get_program(has_wv2=not wv2_is_eye)

    support = inputs["support_features"].astype(np.float32)
    query = inputs["query_features"].astype(np.float32)
    in_maps = []
    for c in range(N_CORES):
        m = dict(w)
        m["sup_x"] = np.ascontiguousarray(
            support[c * BL:(c + 1) * BL].reshape(NSEQ, S, E))
        m["qf"] = np.ascontiguousarray(query[c * BL:(c + 1) * BL])
        in_maps.append(m)

    res = run_bass_kernel_spmd(nc, in_maps, list(range(N_CORES)))
    out = np.concatenate([res.results[c]["out"] for c in range(N_CORES)], axis=0)
    return out.reshape(B, 196, E).astype(np.float32)


# revision 13
# speedup vs baseline: 1.8700x; 1.1734x over previous
"""Trainium2 Bass kernel for nn_PromptGenerator (sparse_attention).

Contract: kernel(**inputs) takes FULL inputs (as produced by
reference.setup_inputs) and returns the FULL [32, 196, 1024] f32 output.
Internally shards batch B=32 across 8 NeuronCores (4 per core), weights
replicated.

Math refactoring (exact, value-independent):
  - LayerNorm affine (ln_w, ln_b) and the in-proj bias are folded into the
    QKV weight/bias on the host; the kernel only standardizes (x-mu)*rsqrt.
  - The attention 1/sqrt(128) scale is folded into the q-part of the in-proj
    weight on the host.
  - Stage-2 never materializes k/v1/v2 over the 1225 support tokens:
      score = (q @ Wk) @ s_img^T          [q @ Wk via A = Wq^T Wk]
      o1    = (attn @ s_img) @ Wv1^T      [softmax rows sum to 1]
      o2    = (attn @ s_mask) @ Wv2^T
    and when Wv2 == I (always true for this module), o2 = attn @ s_mask.
  - All bias vectors in this module are zero (asserted after host folding);
    the kernel specializes on that.

Layouts: q,k projections and attention scores are computed feature-major
(via bf16 DMA-transposes of the standardized activations); v is computed
token-major so that attention output lands feature-major without any PE
transposes.  Softmax row sums come for free from the Exp activation's
accum_out; normalization is applied as a per-partition scale where the
token dim sits on partitions.
"""

import sys

sys.path.insert(0, "/opt/trn_rl_repo")

import numpy as np
import ml_dtypes

import concourse.bass as bass
import concourse.mybir as mybir
import concourse.tile as tile
from concourse.bass_utils import run_bass_kernel_spmd
from concourse.masks import make_identity

F32 = mybir.dt.float32
BF16 = mybir.dt.bfloat16
AF = mybir.ActivationFunctionType
BF = ml_dtypes.bfloat16

N_CORES = 8
B = 32
BL = B // N_CORES          # batches per core = 4
N = 25                     # support shots
S = 98                     # support seq len
SP = 112                   # S padded to /16 for dma-transpose
E = 1024
H = 8
KT = E // 128              # 8 k-tiles over embed dim
SB = 4                     # seqs per support block
NSEQ = BL * N              # 100 seqs per core
NBLK = NSEQ // SB          # 25 blocks
XW = SB * SP               # 448: feature-major width (112 stride, 32B-aligned)
SQ = 49                    # query img tokens
SQP = 64                   # padded
QW = BL * SQP              # 256
NIJ = N * SQ               # 1225
NIJT = 10                  # nij tiles of 128 (last = 73)
LAST_NIJ = NIJ - 9 * 128   # 73
LAST_NIJP = 80             # padded to /16
EPS = 1e-5


# ------------------------------------------------------------- wait splitting
def _split_multi_waits(nc, max_waits=1):
    """walrus in this env rejects instructions carrying more than one sync
    wait.  Tile's semaphore assignment freely attaches several.  Hoist the
    extra waits onto single-wait NoOps on the same engine, inserted right
    before the instruction (the engine's NX processes its stream in order,
    so this is semantics-preserving)."""
    n_split = 0
    for fn in nc.m.functions:
        for bb in fn.blocks:
            insts = list(bb.instructions)
            need = any(
                i.sync_info is not None and len(i.sync_info.on_wait) > max_waits
                for i in insts)
            if not need:
                continue
            new = []
            for inst in insts:
                si = inst.sync_info
                if si is not None and len(si.on_wait) > max_waits:
                    waits = list(si.on_wait)
                    extra, keep = waits[:-max_waits], waits[-max_waits:]
                    for j, w in enumerate(extra):
                        nop = mybir.InstNoOp(
                            name=f"{inst.name}-w{j}",
                            engine=inst.engine,
                            bass_nofuse=True,
                            sync_info=mybir.SyncInfo(on_wait=[w], on_update=[]),
                        )
                        nc.register_instruction(nop)
                        new.append(nop)
                    inst.sync_info = mybir.SyncInfo(
                        on_wait=keep, on_update=list(si.on_update))
                    n_split += 1
                new.append(inst)
            bb.instructions = new
    return n_split


# ---------------------------------------------------------------- host prep
def _fold_ln_inproj(ln_w, ln_b, in_w, in_b, q_scale):
    """qkv = ln(x) @ in_w.T + in_b  ->  xh @ W1 + c1 with xh standardized."""
    W1 = (ln_w[:, None] * in_w.T).astype(np.float32)          # [E, 3E]
    c1 = (ln_b @ in_w.T + in_b).astype(np.float32)            # [3E]
    W1[:, :E] *= q_scale
    c1[:E] *= q_scale
    return W1, c1


def _ktiles(w):  # [E, X] f32 -> [KT, 128, X] bf16
    return np.ascontiguousarray(w.reshape(KT, 128, -1)).astype(BF)


def _gaussian_bank_np(sigma=1.0):
    x = np.arange(7.0)
    xx, yy = np.meshgrid(x, x, indexing="ij")
    cy = np.arange(7.0)[:, None, None, None]
    cx = np.arange(7.0)[None, :, None, None]
    k = np.exp(-((xx[None, None] - cy) ** 2 + (yy[None, None] - cx) ** 2)
               / (2.0 * sigma ** 2))
    k = k / k.sum(axis=(-2, -1), keepdims=True)
    return k.reshape(49, 49).astype(np.float32)               # [q, ij]


def _prep_weights(inp):
    qs = np.float32(1.0 / np.sqrt(128.0))
    W1s, c1s = _fold_ln_inproj(inp["ln_w"], inp["ln_b"], inp["s_in_w"], inp["s_in_b"], qs)
    W1q, c1q = _fold_ln_inproj(inp["ln_w"], inp["ln_b"], inp["q_in_w"], inp["q_in_b"], qs)
    A = (inp["Wq"].T @ inp["Wk"]).astype(np.float32)          # [E, E]
    g = _gaussian_bank_np(1.0)
    gfull = np.tile(g, (1, N)).astype(np.float32)             # [49, 1225]
    wv2_is_eye = bool(np.array_equal(inp["Wv2"], np.eye(E, dtype=inp["Wv2"].dtype)))

    # all bias-like terms must be zero for this kernel specialization
    for z in (c1s, c1q, inp["s_out_b"], inp["q_out_b"], inp["bq"], inp["bk"],
              inp["bv1"], inp["bv2"]):
        assert not np.any(np.asarray(z)), "nonzero-bias inputs not supported"

    w = {
        "w1s_qk": _ktiles(W1s[:, :2 * E]),
        "w1s_v": _ktiles(W1s[:, 2 * E:]),
        "w2s": _ktiles(inp["s_out_w"].T.astype(np.float32)),
        "w1q_qk": _ktiles(W1q[:, :2 * E]),
        "w1q_v": _ktiles(W1q[:, 2 * E:]),
        "w2q": _ktiles(inp["q_out_w"].T.astype(np.float32)),
        "a_mat": _ktiles(A),
        "wv1": _ktiles(inp["Wv1"].T.astype(np.float32)),
        "gmask": gfull,
    }
    if not wv2_is_eye:
        w["wv2"] = _ktiles(inp["Wv2"].T.astype(np.float32))
    return w, wv2_is_eye


# ---------------------------------------------------------------- builder
def build_program(has_wv2):
    nc = bass.Bass()

    sup_x = nc.declare_dram_parameter("sup_x", [NSEQ, S, E], F32, isOutput=False)
    qf = nc.declare_dram_parameter("qf", [BL, S, E], F32, isOutput=False)
    w1s_qk = nc.declare_dram_parameter("w1s_qk", [KT, 128, 2 * E], BF16, isOutput=False)
    w1s_v = nc.declare_dram_parameter("w1s_v", [KT, 128, E], BF16, isOutput=False)
    w2s = nc.declare_dram_parameter("w2s", [KT, 128, E], BF16, isOutput=False)
    w1q_qk = nc.declare_dram_parameter("w1q_qk", [KT, 128, 2 * E], BF16, isOutput=False)
    w1q_v = nc.declare_dram_parameter("w1q_v", [KT, 128, E], BF16, isOutput=False)
    w2q = nc.declare_dram_parameter("w2q", [KT, 128, E], BF16, isOutput=False)
    a_mat = nc.declare_dram_parameter("a_mat", [KT, 128, E], BF16, isOutput=False)
    wv1 = nc.declare_dram_parameter("wv1", [KT, 128, E], BF16, isOutput=False)
    gmask = nc.declare_dram_parameter("gmask", [SQ, NIJ], F32, isOutput=False)
    wv2 = None
    if has_wv2:
        wv2 = nc.declare_dram_parameter("wv2", [KT, 128, E], BF16, isOutput=False)

    out = nc.declare_dram_parameter("out", [BL, 196, E], F32, isOutput=True)
    # scratch in DRAM (declared as outputs: useful for debugging, cheap)
    sup_img = nc.declare_dram_parameter("sup_img", [NSEQ * SQ, E], BF16, isOutput=True)
    sup_mask = nc.declare_dram_parameter("sup_mask", [NSEQ * SQ, E], BF16, isOutput=True)

    with tile.TileContext(nc) as tc:
        with tc.tile_pool(name="const", bufs=1) as cpool:
            eps_sb = cpool.tile([128, 1], F32, tag="eps")
            nc.gpsimd.memset(eps_sb, EPS)
            ident = cpool.tile([128, 128], BF16, tag="ident")
            make_identity(nc, ident)

            _support_phase(nc, tc, sup_x, w1s_qk, w1s_v, w2s, sup_img, sup_mask,
                           eps_sb, ident)
            with tc.tile_pool(name="qpT_pool", bufs=1) as qpT_pool:
                qpT = _query_phase(nc, tc, qf, w1q_qk, w1q_v, w2q, out, eps_sb,
                                   qpT_pool)
                _stage2_phase(nc, tc, qpT, a_mat, wv1, wv2, gmask,
                              sup_img, sup_mask, out, ident)

    _split_multi_waits(nc)
    nc.finalize()
    return nc


def _ln_standardize(nc, pool_small, x_t, xh_t, nrows, eps_sb):
    """xh[:nrows] = (x - mean) * rsqrt(var + eps), bf16 out."""
    stats = pool_small.tile([128, 2, 6], F32, tag="bnst")
    nc.vector.bn_stats(stats[:nrows, 0, :], x_t[:nrows, 0:512])
    nc.vector.bn_stats(stats[:nrows, 1, :], x_t[:nrows, 512:1024])
    mv = pool_small.tile([128, 2], F32, tag="bnmv")
    nc.vector.bn_aggr(mv[:nrows], stats[:nrows])
    lnv = pool_small.tile([128, 1], F32, tag="lnv")
    nc.scalar.activation(lnv[:nrows], mv[:nrows, 1:2], AF.Ln, bias=eps_sb[:nrows])
    r = pool_small.tile([128, 1], F32, tag="rstd")
    nc.scalar.activation(r[:nrows], lnv[:nrows], AF.Exp, scale=-0.5)
    nmu = pool_small.tile([128, 1], F32, tag="nmu")
    nc.vector.tensor_scalar_mul(nmu[:nrows], mv[:nrows, 0:1], -1.0)
    nc.vector.tensor_scalar(
        out=xh_t[:nrows], in0=x_t[:nrows], scalar1=nmu[:nrows], scalar2=r[:nrows],
        op0=mybir.AluOpType.add, op1=mybir.AluOpType.mult)


def _support_phase(nc, tc, sup_x, w1s_qk, w1s_v, w2s, sup_img, sup_mask, eps_sb,
                   ident):
    with (
        tc.tile_pool(name="wa", bufs=1) as wpool,
        tc.tile_pool(name="a_x", bufs=8) as x_pool,
        tc.tile_pool(name="a_small", bufs=8) as small,
        tc.tile_pool(name="a_xh", bufs=6) as xh_pool,
        tc.tile_pool(name="a_xhT", bufs=2) as xhT_pool,
        tc.tile_pool(name="a_qkT", bufs=2) as qkT_pool,
        tc.tile_pool(name="a_v", bufs=6) as v_pool,
        tc.tile_pool(name="a_en", bufs=10) as en_pool,
        tc.tile_pool(name="a_eT", bufs=10) as eT_pool,
        tc.tile_pool(name="a_oT", bufs=2) as oT_pool,
        tc.tile_pool(name="a_sup", bufs=4) as sup_pool,
        tc.tile_pool(name="a_ps_big", bufs=2, space="PSUM") as ps_big,
        tc.tile_pool(name="a_ps_l", bufs=2, space="PSUM") as ps_l_pool,
        tc.tile_pool(name="a_ps_t", bufs=2, space="PSUM") as ps_t_pool,
        tc.tile_pool(name="a_ps_o", bufs=2, space="PSUM") as ps_o_pool,
    ):
        w1qk_sb, w1v_sb, w2_sb = [], [], []
        for k in range(KT):
            t = wpool.tile([128, 2 * E], BF16, tag=f"w1qk{k}")
            nc.sync.dma_start(t, w1s_qk[k])
            w1qk_sb.append(t)
            t = wpool.tile([128, E], BF16, tag=f"w1v{k}")
            nc.sync.dma_start(t, w1s_v[k])
            w1v_sb.append(t)
            t = wpool.tile([128, E], BF16, tag=f"w2{k}")
            nc.sync.dma_start(t, w2s[k])
            w2_sb.append(t)

        for blk in range(NBLK):
            xs, xhs = [], []
            for s in range(SB):
                seq = blk * SB + s
                x_t = x_pool.tile([S, E], F32, tag="x")
                xsrc = sup_x[seq].rearrange("(i j) f -> i j f", j=14)
                nc.sync.dma_start(x_t[0:SQ], xsrc[:, 0:7, :])
                nc.sync.dma_start(x_t[SQ:S], xsrc[:, 7:14, :])
                xs.append(x_t)
                xh_t = xh_pool.tile([SP, E], BF16, tag="xh")
                nc.gpsimd.memset(xh_t[96:SP, :], 0.0)
                _ln_standardize(nc, small, x_t, xh_t, S, eps_sb)
                xhs.append(xh_t)

            # feature-major standardized activations, [KT][128, XW] (112-stride)
            xhT = [xhT_pool.tile([128, XW], BF16, tag=f"xhT{k}", name=f"xhT{k}")
                   for k in range(KT)]
            for k in range(KT):
                eng = nc.sync if k % 2 == 0 else nc.scalar
                for s in range(SB):
                    eng.dma_start_transpose(
                        out=xhT[k][:, s * SP: s * SP + SP],
                        in_=xhs[s][:, k * 128:(k + 1) * 128])

            # q,k projections, feature-major [16][128, XW]
            qkT = []
            for f in range(2 * KT):
                qt = qkT_pool.tile([128, XW], BF16, tag=f"qkT{f}")
                ps = ps_big.tile([128, XW], F32, tag="big")
                for k in range(KT):
                    nc.tensor.matmul(
                        ps, lhsT=w1qk_sb[k][:, f * 128:(f + 1) * 128],
                        rhs=xhT[k], start=(k == 0), stop=(k == KT - 1))
                nc.scalar.copy(qt, ps)
                qkT.append(qt)

            # v projection, token-major per seq [SB][S, E]
            vs = []
            for s in range(SB):
                v_t = v_pool.tile([S, E], BF16, tag="v")
                for half in range(2):
                    ps = ps_big.tile([S, 512], F32, tag="big")
                    for k in range(KT):
                        nc.tensor.matmul(
                            ps, lhsT=xhT[k][:, s * SP: s * SP + S],
                            rhs=w1v_sb[k][:, half * 512:(half + 1) * 512],
                            start=(k == 0), stop=(k == KT - 1))
                    nc.scalar.copy(v_t[:, half * 512:(half + 1) * 512], ps)
                vs.append(v_t)

            # attention, software-pipelined over seqs so PE never waits on
            # the exp -> normalize -> transpose chain.  Logits are batched 4
            # heads per PSUM bank so one Exp covers 4 heads; rowsums via DVE
            # reduce.  The out-proj is interleaved two seqs behind to keep PE
            # duty high through the ACT-bound attention region.
            oT = [oT_pool.tile([128, SB * S], BF16, tag=f"oT{h}", name=f"oT{h}")
                  for h in range(H)]

            def emit_logits(s):
                en4s = []
                for g in range(2):
                    ps_l = ps_l_pool.tile([S, 4 * S], F32, tag="psl")
                    for hh in range(4):
                        h = g * 4 + hh
                        nc.tensor.matmul(
                            ps_l[:, hh * S:(hh + 1) * S],
                            lhsT=qkT[h][:, s * SP: s * SP + S],
                            rhs=qkT[KT + h][:, s * SP: s * SP + S],
                            skip_group_check=True)
                    en4 = en_pool.tile([S, 4 * S], BF16, tag="en")
                    nc.scalar.activation(en4, ps_l, AF.Exp)
                    racc = small.tile([S, 4], F32, tag="racc")
                    nc.vector.reduce_sum(
                        racc, en4.rearrange("p (h j) -> p h j", h=4),
                        axis=mybir.AxisListType.X)
                    rr = small.tile([S, 4], F32, tag="rr")
                    nc.vector.reciprocal(rr, racc)
                    for hh in range(4):
                        nc.vector.tensor_scalar_mul(
                            en4[:, hh * S:(hh + 1) * S],
                            en4[:, hh * S:(hh + 1) * S], rr[:, hh:hh + 1])
                    en4s.append(en4)
                return en4s

            def emit_o(s, en4s):
                eTs = []
                for h in range(H):
                    en = en4s[h // 4][:, (h % 4) * S:(h % 4 + 1) * S]
                    ps_t = ps_t_pool.tile([S, S], BF16, tag="pst")
                    nc.tensor.transpose(ps_t, en, ident[:S, :S])
                    eT = eT_pool.tile([S, S], BF16, tag="eT")
                    nc.vector.tensor_copy(eT, ps_t)
                    eTs.append(eT)
                for h in range(H):
                    ps_o = ps_o_pool.tile([128, S], F32, tag="pso")
                    nc.tensor.matmul(ps_o, lhsT=vs[s][:, h * 128:(h + 1) * 128],
                                     rhs=eTs[h])
                    nc.vector.tensor_copy(oT[h][:, s * S: s * S + S], ps_o)

            def emit_outproj(s):
                seq = blk * SB + s
                sup_bf = sup_pool.tile([S, E], BF16, tag="supbf")
                for half in range(2):
                    ps = ps_big.tile([S, 512], F32, tag="big")
                    for e in range(KT):
                        nc.tensor.matmul(
                            ps, lhsT=oT[e][:, s * S: s * S + S],
                            rhs=w2_sb[e][:, half * 512:(half + 1) * 512],
                            start=(e == 0), stop=(e == KT - 1))
                    nc.vector.tensor_add(
                        sup_bf[:, half * 512:(half + 1) * 512], ps,
                        xs[s][:, half * 512:(half + 1) * 512])
                nc.sync.dma_start(
                    out=sup_img[seq * SQ:(seq + 1) * SQ], in_=sup_bf[0:SQ])
                nc.sync.dma_start(
                    out=sup_mask[seq * SQ:(seq + 1) * SQ], in_=sup_bf[SQ:S])

            prev = None
            for s in range(SB):
                cur = emit_logits(s)
                if prev is not None:
                    emit_o(s - 1, prev)
                if s >= 2:
                    emit_outproj(s - 2)
                prev = cur
            emit_o(SB - 1, prev)
            emit_outproj(SB - 2)
            emit_outproj(SB - 1)


def _query_phase(nc, tc, qf, w1q_qk, w1q_v, w2q, out, eps_sb, qpT_pool):
    """Query-image self-attention.  Returns qpT (feature-major bf16 tiles)."""
    with (
        tc.tile_pool(name="wb", bufs=1) as wpool,
        tc.tile_pool(name="b_x", bufs=4) as x_pool,
        tc.tile_pool(name="b_small", bufs=8) as small,
        tc.tile_pool(name="b_once", bufs=1) as once,
        tc.tile_pool(name="b_xh", bufs=4) as xh_pool,
        tc.tile_pool(name="b_en", bufs=8) as en_pool,
        tc.tile_pool(name="b_eT", bufs=8) as eT_pool,
        tc.tile_pool(name="b_v", bufs=4) as v_pool,
        tc.tile_pool(name="b_qp", bufs=4) as qp_pool,
        tc.tile_pool(name="b_ps1", bufs=2, space="PSUM") as ps1,
        tc.tile_pool(name="b_ps2", bufs=2, space="PSUM") as ps2,
        tc.tile_pool(name="b_ps3", bufs=2, space="PSUM") as ps3,
    ):
        w1qk_sb, w1v_sb, w2_sb = [], [], []
        for k in range(KT):
            t = wpool.tile([128, 2 * E], BF16, tag=f"w1qk{k}")
            nc.sync.dma_start(t, w1q_qk[k])
            w1qk_sb.append(t)
            t = wpool.tile([128, E], BF16, tag=f"w1v{k}")
            nc.sync.dma_start(t, w1q_v[k])
            w1v_sb.append(t)
            t = wpool.tile([128, E], BF16, tag=f"w2{k}")
            nc.sync.dma_start(t, w2q[k])
            w2_sb.append(t)

        xqs, xhqs = [], []
        for b in range(BL):
            x_t = x_pool.tile([SQ, E], F32, tag="xq")
            nc.sync.dma_start(
                x_t, qf[b].rearrange("(i j) f -> i j f", j=14)[:, 0:7, :])
            xqs.append(x_t)
            xh_t = xh_pool.tile([SQP, E], BF16, tag="xhq")
            nc.gpsimd.memset(xh_t[32:SQP, :], 0.0)
            _ln_standardize(nc, small, x_t, xh_t, SQ, eps_sb)
            xhqs.append(xh_t)

        xhT = [once.tile([128, QW], BF16, tag=f"xhqT{k}", name=f"xhqT{k}")
               for k in range(KT)]
        for b in range(BL):
            for k in range(KT):
                nc.sync.dma_start_transpose(
                    out=xhT[k][:, b * SQP:(b + 1) * SQP],
                    in_=xhqs[b][:, k * 128:(k + 1) * 128])

        qkT = []
        for f in range(2 * KT):
            ps = ps1.tile([128, QW], F32, tag="psq1")
            for k in range(KT):
                nc.tensor.matmul(ps, lhsT=w1qk_sb[k][:, f * 128:(f + 1) * 128],
                                 rhs=xhT[k], start=(k == 0), stop=(k == KT - 1))
            qt = once.tile([128, QW], BF16, tag=f"qkTq{f}")
            nc.scalar.copy(qt, ps)
            qkT.append(qt)

        vqs = []
        for b in range(BL):
            v_t = v_pool.tile([SQ, E], BF16, tag="vq")
            for half in range(2):
                ps = ps2.tile([SQ, 512], F32, tag="psq2")
                for k in range(KT):
                    nc.tensor.matmul(
                        ps, lhsT=xhT[k][:, b * SQP: b * SQP + SQ],
                        rhs=w1v_sb[k][:, half * 512:(half + 1) * 512],
                        start=(k == 0), stop=(k == KT - 1))
                nc.scalar.copy(v_t[:, half * 512:(half + 1) * 512], ps)
            vqs.append(v_t)

        oT = [once.tile([128, BL * SQ], BF16, tag=f"oTq{h}", name=f"oTq{h}")
              for h in range(H)]
        for b in range(BL):
            for h in range(H):
                ps_l = ps3.tile([128, SQ], F32, tag="psq3")
                nc.tensor.matmul(ps_l[:SQ, :],
                                 lhsT=qkT[h][:, b * SQP: b * SQP + SQ],
                                 rhs=qkT[KT + h][:, b * SQP: b * SQP + SQ])
                en = en_pool.tile([SQP, 128], BF16, tag="enq")
                racc = small.tile([128, 1], F32, tag="raccq")
                nc.scalar.activation(en[:SQ, 0:SQ], ps_l[:SQ, :], AF.Exp,
                                     accum_out=racc[:SQ])
                rr = small.tile([128, 1], F32, tag="rrq")
                nc.vector.reciprocal(rr[:SQ], racc[:SQ])
                nc.vector.tensor_scalar_mul(en[:SQ, 0:SQ], en[:SQ, 0:SQ], rr[:SQ])
                eT = eT_pool.tile([128, SQP], BF16, tag="eTq")
                nc.sync.dma_start_transpose(out=eT, in_=en)
                ps_o = ps3.tile([128, SQ], F32, tag="psq3")
                nc.tensor.matmul(ps_o, lhsT=vqs[b][:, h * 128:(h + 1) * 128],
                                 rhs=eT[:SQ, 0:SQ])
                nc.vector.tensor_copy(oT[h][:, b * SQ:(b + 1) * SQ], ps_o)

        qps = []
        for b in range(BL):
            qp = qp_pool.tile([SQ, E], F32, tag="qp")
            for half in range(2):
                ps = ps2.tile([SQ, 512], F32, tag="psq2")
                for e in range(KT):
                    nc.tensor.matmul(
                        ps, lhsT=oT[e][:, b * SQ:(b + 1) * SQ],
                        rhs=w2_sb[e][:, half * 512:(half + 1) * 512],
                        start=(e == 0), stop=(e == KT - 1))
                nc.vector.tensor_add(qp[:, half * 512:(half + 1) * 512], ps,
                                     xqs[b][:, half * 512:(half + 1) * 512])
            qps.append(qp)
            # output: query_tokens img half
            nc.sync.dma_start(
                out=out[b, 98:196].rearrange("(i j) f -> i j f", j=14)[:, 0:7, :],
                in_=qp)
            # query_tokens mask half: copy input -> output via SBUF bounce
            qm = x_pool.tile([SQ, E], F32, tag="qm")
            nc.sync.dma_start(
                qm, qf[b].rearrange("(i j) f -> i j f", j=14)[:, 7:14, :])
            nc.sync.dma_start(
                out=out[b, 98:196].rearrange("(i j) f -> i j f", j=14)[:, 7:14, :],
                in_=qm)

        # qp cast + transpose for stage 2
        qpT = [qpT_pool.tile([128, QW], BF16, tag=f"qpT{k}", name=f"qpT{k}")
               for k in range(KT)]
        for b in range(BL):
            qpb = xh_pool.tile([SQP, E], BF16, tag="qpb")
            nc.gpsimd.memset(qpb[32:SQP, :], 0.0)
            nc.vector.tensor_copy(qpb[:SQ], qps[b])
            for k in range(KT):
                nc.sync.dma_start_transpose(
                    out=qpT[k][:, b * SQP:(b + 1) * SQP],
                    in_=qpb[:, k * 128:(k + 1) * 128])
        return qpT


def _stage2_phase(nc, tc, qpT, a_mat, wv1, wv2, gmask, sup_img, sup_mask, out,
                  ident):
    with (
        tc.tile_pool(name="wc", bufs=1) as wpool,
        tc.tile_pool(name="c_simg", bufs=14) as simg_pool,
        tc.tile_pool(name="c_smask", bufs=14) as smask_pool,
        tc.tile_pool(name="c_sT", bufs=2) as sT_pool,
        tc.tile_pool(name="c_small", bufs=6) as small,
        tc.tile_pool(name="c_e2", bufs=2) as e2_pool,
        tc.tile_pool(name="c_e2T", bufs=12) as e2T_pool,
        tc.tile_pool(name="c_o", bufs=9) as o_pool,
        tc.tile_pool(name="c_ps_sc", bufs=2, space="PSUM") as ps_sc,
        tc.tile_pool(name="c_ps_op", bufs=2, space="PSUM") as ps_op,
        tc.tile_pool(name="c_ps_o1t", bufs=2, space="PSUM") as ps_o1tp,
        tc.tile_pool(name="c_ps_ct", bufs=2, space="PSUM") as ps_ct,
    ):
        a_sb, wv1_sb, wv2_sb = [], [], []
        for k in range(KT):
            t = wpool.tile([128, E], BF16, tag=f"a{k}")
            nc.sync.dma_start(t, a_mat[k])
            a_sb.append(t)
            t = wpool.tile([128, E], BF16, tag=f"wv1{k}")
            nc.sync.dma_start(t, wv1[k])
            wv1_sb.append(t)
            if wv2 is not None:
                t = wpool.tile([128, E], BF16, tag=f"wv2{k}")
                nc.sync.dma_start(t, wv2[k])
                wv2_sb.append(t)
        gm_sb = wpool.tile([SQ, NIJ], F32, tag="gm")
        nc.sync.dma_start(gm_sb, gmask[:])

        # qk2 = qp @ A, feature-major [KT][128, QW]
        qk2T = []
        for eo in range(KT):
            ps = ps_sc.tile([128, 512], F32, tag="pssc")
            for ei in range(KT):
                nc.tensor.matmul(ps[:, :QW],
                                 lhsT=a_sb[ei][:, eo * 128:(eo + 1) * 128],
                                 rhs=qpT[ei], start=(ei == 0), stop=(ei == KT - 1))
            qt = wpool.tile([128, QW], BF16, tag=f"qk2T{eo}")
            nc.scalar.copy(qt, ps[:, :QW])
            qk2T.append(qt)

        nij_rows = [128] * 9 + [LAST_NIJ]
        chunks = [(0, 512), (512, 512), (1024, NIJ - 1024)]
        for b in range(BL):
            simg_t, smask_t = [], []
            for t_i in range(NIJT):
                rows = nij_rows[t_i]
                prows = 128 if t_i < 9 else LAST_NIJP
                st = simg_pool.tile([prows, E], BF16, tag="simg")
                if t_i == 9:
                    nc.gpsimd.memset(st[64:prows, :], 0.0)
                nc.sync.dma_start(
                    st[:rows],
                    sup_img[b * NIJ + t_i * 128: b * NIJ + t_i * 128 + rows])
                simg_t.append(st)
                st = smask_pool.tile([prows, E], BF16, tag="smask")
                nc.sync.dma_start(
                    st[:rows],
                    sup_mask[b * NIJ + t_i * 128: b * NIJ + t_i * 128 + rows])
                smask_t.append(st)

            # s_imgT feature-major [KT][128, 1280] via PE transposes
            sT = [sT_pool.tile([128, NIJT * 128], BF16, tag=f"sT{k}", name=f"sT{k}")
                  for k in range(KT)]
            for t_i in range(NIJT):
                rows = nij_rows[t_i]
                for k in range(KT):
                    ps_t = ps_ct.tile([128, 128], BF16, tag="psct")
                    nc.tensor.transpose(ps_t[:, :rows], simg_t[t_i][:rows, k * 128:(k + 1) * 128],
                                        ident[:rows, :rows])
                    nc.vector.tensor_copy(
                        sT[k][:, t_i * 128: t_i * 128 + rows], ps_t[:, :rows])

            # score chunks -> gaussian mask -> exp -> e2 (unnormalized attn)
            e2 = e2_pool.tile([SQP, NIJT * 128], BF16, tag="e2")
            racc3 = small.tile([SQ, 3], F32, tag="racc3")
            for ci, (c0, cw) in enumerate(chunks):
                ps = ps_sc.tile([128, 512], F32, tag="pssc")
                for k in range(KT):
                    nc.tensor.matmul(ps[:SQ, :cw],
                                     lhsT=qk2T[k][:, b * SQP: b * SQP + SQ],
                                     rhs=sT[k][:, c0: c0 + cw],
                                     start=(k == 0), stop=(k == KT - 1))
                msk = small.tile([SQ, 512], F32, tag="msk")
                nc.vector.tensor_mul(msk[:, :cw], ps[:SQ, :cw], gm_sb[:, c0:c0 + cw])
                nc.scalar.activation(e2[:SQ, c0:c0 + cw], msk[:, :cw], AF.Exp,
                                     accum_out=racc3[:, ci:ci + 1])
            r2 = small.tile([SQ, 1], F32, tag="r2")
            nc.vector.reduce_sum(r2, racc3, axis=mybir.AxisListType.X)
            rr2 = small.tile([SQ, 1], F32, tag="rr2")
            nc.vector.reciprocal(rr2, r2)

            e2T = []
            for t_i in range(NIJT):
                et = e2T_pool.tile([128, SQP], BF16, tag="e2T")
                nc.scalar.dma_start_transpose(
                    out=et, in_=e2[:, t_i * 128:(t_i + 1) * 128])
                e2T.append(et)

            def _emit_o(src_tiles, proj_sb, col0):
                """o = ((attn @ src) [@ W^T]) * rr2 -> out[b, support rows,
                img cols (col0=0) or mask cols (col0=7)]."""
                if proj_sb is None:
                    for half in range(2):
                        ps = ps_op.tile([SQ, 512], F32, tag="psop")
                        for t_i in range(NIJT):
                            rows = nij_rows[t_i]
                            nc.tensor.matmul(
                                ps, lhsT=e2T[t_i][:rows, 0:SQ],
                                rhs=src_tiles[t_i][:rows,
                                                   half * 512:(half + 1) * 512],
                                start=(t_i == 0), stop=(t_i == NIJT - 1))
                        osb = o_pool.tile([SQ, 512], F32, tag="osb")
                        nc.scalar.activation(osb, ps, AF.Copy, scale=rr2)
                        nc.sync.dma_start(
                            out=out[b, 0:98].rearrange(
                                "(i j) f -> i j f", j=14)[
                                :, col0:col0 + 7, half * 512:(half + 1) * 512],
                            in_=osb)
                else:
                    oT_sb = []
                    for e in range(KT):
                        ps = ps_o1tp.tile([128, SQ], F32, tag="pso1t")
                        for t_i in range(NIJT):
                            rows = nij_rows[t_i]
                            nc.tensor.matmul(
                                ps, lhsT=src_tiles[t_i][:rows,
                                                        e * 128:(e + 1) * 128],
                                rhs=e2T[t_i][:rows, 0:SQ],
                                start=(t_i == 0), stop=(t_i == NIJT - 1))
                        ot = o_pool.tile([128, SQ], BF16, tag="opt")
                        nc.scalar.copy(ot, ps)
                        oT_sb.append(ot)
                    for half in range(2):
                        ps = ps_op.tile([SQ, 512], F32, tag="psop")
                        for e in range(KT):
                            nc.tensor.matmul(
                                ps, lhsT=oT_sb[e],
                                rhs=proj_sb[e][:, half * 512:(half + 1) * 512],
                                start=(e == 0), stop=(e == KT - 1))
                        osb = o_pool.tile([SQ, 512], F32, tag="osb")
                        nc.scalar.activation(osb, ps, AF.Copy, scale=rr2)
                        nc.sync.dma_start(
                            out=out[b, 0:98].rearrange(
                                "(i j) f -> i j f", j=14)[
                                :, col0:col0 + 7, half * 512:(half + 1) * 512],
                            in_=osb)

            _emit_o(simg_t, wv1_sb, 0)
            _emit_o(smask_t, wv2_sb if wv2 is not None else None, 7)


# ---------------------------------------------------------------- entry point
_CACHE = {}


def _get_program(has_wv2):
    if has_wv2 not in _CACHE:
        _CACHE[has_wv2] = build_program(has_wv2)
    return _CACHE[has_wv2]


def kernel(**inputs):
    inputs = {k: np.asarray(v) for k, v in inputs.items()}
    w, wv2_is_eye = _prep_weights(inputs)
    nc = _get_program(has_wv2=not wv2_is_eye)

    support = inputs["support_features"].astype(np.float32)
    query = inputs["query_features"].astype(np.float32)
    in_maps = []
    for c in range(N_CORES):
        m = dict(w)
        m["sup_x"] = np.ascontiguousarray(
            support[c * BL:(c + 1) * BL].reshape(NSEQ, S, E))
        m["qf"] = np.ascontiguousarray(query[c * BL:(c + 1) * BL])
        in_maps.append(m)

    res = run_bass_kernel_spmd(nc, in_maps, list(range(N_CORES)))
    out = np.concatenate([res.results[c]["out"] for c in range(N_CORES)], axis=0)
    return out.reshape(B, 196, E).astype(np.float32)
